# revision 1
# baseline (speedup 1.0000x reference)
"""MQA (GQA with 1 KV group) attention kernel for 8 Trainium2 NeuronCores.

Sharding: core c -> batch b = c//4, head-group hg = c%4 (4 of 16 query heads).
Each core computes Q/K/V projections from x[b]^T, causal attention for its 4
heads in transposed layout (S^T[kv, q] tiles), and a partial output
projection out_partial = A_h @ Wo[:, cols_h]^T.  Host sums the 4 partials per
batch and adds bo.

Matmul operands are bf16 (PSUM accumulation is f32); f32 matmuls lower to two
PE passes on trn2, so bf16 halves tensor-engine time and input DMA.  The
causal mask is hardcoded structurally (upper-triangular tiles skipped; 4
precomputed diagonal-block masks), the padding mask enters as a per-kv-
partition bias fused into the exp activation.  Softmax row-sums accumulate on
the PE via a ones-column matmul into a dedicated PSUM bank.
"""

import sys

sys.path.insert(0, "/opt/trn_rl_repo")

import ml_dtypes
import numpy as np

import concourse.bass as bass
import concourse.tile as tile
from concourse import bacc
from concourse import mybir
from concourse.bass import ts
from concourse.bass_utils import run_bass_kernel_spmd
from concourse.masks import make_identity

B, S, HID = 2, 2048, 2048
H, D = 16, 128
HPC = 4              # heads per core
DPH = HPC * D        # 512: head dims per core
NCORES = 8
SC = 512             # s-chunk (free dim for most matmuls)
NSC = S // SC        # 4
NT = S // 128        # 16 128-tiles along s / hid
NHT = HID // 128     # 16 hid tiles
SCALE = 1.0 / float(np.sqrt(D))
NEG = -1.0e9

F32 = mybir.dt.float32
BF16 = mybir.dt.bfloat16
NP_BF16 = ml_dtypes.bfloat16

_PROGRAM = None
LAST_RESULT = None


def _build_program():
    nc = bacc.Bacc()
    xT = nc.declare_dram_parameter("xT", [HID, S], BF16, isOutput=False)
    wq = nc.declare_dram_parameter("wq", [HID, DPH], BF16, isOutput=False)
    wk = nc.declare_dram_parameter("wk", [HID, D], BF16, isOutput=False)
    wv = nc.declare_dram_parameter("wv", [HID, D], BF16, isOutput=False)
    wo = nc.declare_dram_parameter("wo", [DPH, HID], BF16, isOutput=False)
    bq = nc.declare_dram_parameter("bq", [128, HPC], F32, isOutput=False)
    bkv = nc.declare_dram_parameter("bkv", [128, 2], F32, isOutput=False)
    padb = nc.declare_dram_parameter("padb", [128, NT], F32, isOutput=False)
    dmask = nc.declare_dram_parameter("dmask", [128, 4 * SC], F32, isOutput=False)
    out = nc.declare_dram_parameter("out", [S, HID], F32, isOutput=True)

    Exp = mybir.ActivationFunctionType.Exp
    Ident = mybir.ActivationFunctionType.Identity

    with tile.TileContext(nc) as tc:
        with (
            tc.tile_pool(name="consts", bufs=1) as consts,
            tc.tile_pool(name="persist", bufs=1) as persist,
        ):
            ident = consts.tile([128, 128], BF16)
            make_identity(nc, ident[:])
            ones_col = consts.tile([128, 1], F32)
            nc.vector.memset(ones_col[:], 1.0)
            ones_row = consts.tile([1, 128], F32)
            nc.vector.memset(ones_row[:], 1.0)
            bq_sb = consts.tile([128, HPC], F32)
            nc.sync.dma_start(bq_sb[:], bq[:])
            bkv_sb = consts.tile([128, 2], F32)
            nc.sync.dma_start(bkv_sb[:], bkv[:])
            padb_sb = consts.tile([128, NT], F32)
            nc.sync.dma_start(padb_sb[:], padb[:])
            dmask_sb = consts.tile([128, 4 * SC], F32)
            nc.sync.dma_start(dmask_sb[:], dmask[:])

            # Persistent activations (live across stages)
            QT = persist.tile([128, HPC, S], BF16)   # Q^T per head: [d, h, q]
            KT = persist.tile([128, S], BF16)        # K^T: [d, kv]
            V = persist.tile([128, NT, 128], BF16)   # V tiles: [kv_p, kv_tile, d]
            OT = persist.tile([128, HPC, S], BF16)   # (exp(S) V)^T scaled

            # ---------------- Stage 1: projections ----------------
            with (
                tc.tile_pool(name="w1", bufs=1) as w1p,
                tc.tile_pool(name="xt", bufs=20) as xtp,
                tc.tile_pool(name="vt", bufs=2) as vtp,
                tc.tile_pool(name="ps1", bufs=1, space="PSUM") as ps1,
                tc.tile_pool(name="pstr", bufs=2, space="PSUM") as pstr,
            ):
                wq_sb = w1p.tile([128, NHT, DPH], BF16)
                nc.sync.dma_start(
                    wq_sb[:], wq.rearrange("(t p) d -> p t d", p=128)
                )
                wk_sb = w1p.tile([128, NHT, D], BF16)
                nc.sync.dma_start(
                    wk_sb[:], wk.rearrange("(t p) d -> p t d", p=128)
                )
                wv_sb = w1p.tile([128, NHT, D], BF16)
                nc.sync.dma_start(
                    wv_sb[:], wv.rearrange("(t p) d -> p t d", p=128)
                )

                for sc in range(NSC):
                    xts = []
                    for ht in range(NHT):
                        xt_t = xtp.tile([128, SC], BF16, tag="xt")
                        nc.sync.dma_start(
                            xt_t[:], xT[ts(ht, 128), ts(sc, SC)]
                        )
                        xts.append(xt_t)
                    # K^T chunk
                    psk = ps1.tile([128, SC], F32, tag="k")
                    for ht in range(NHT):
                        nc.tensor.matmul(
                            psk[:], wk_sb[:, ht, :], xts[ht][:],
                            start=(ht == 0), stop=(ht == NHT - 1),
                        )
                    nc.scalar.activation(
                        KT[:, ts(sc, SC)], psk[:], Ident, bias=bkv_sb[:, 0:1]
                    )
                    # V^T chunk -> transpose into V tiles
                    psv = ps1.tile([128, SC], F32, tag="v")
                    for ht in range(NHT):
                        nc.tensor.matmul(
                            psv[:], wv_sb[:, ht, :], xts[ht][:],
                            start=(ht == 0), stop=(ht == NHT - 1),
                        )
                    vt_s = vtp.tile([128, SC], BF16, tag="vt")
                    nc.scalar.activation(
                        vt_s[:], psv[:], Ident, bias=bkv_sb[:, 1:2]
                    )
                    for j in range(SC // 128):
                        pst = pstr.tile([128, 128], BF16, tag="tr")
                        nc.tensor.transpose(pst[:], vt_s[:, ts(j, 128)], ident[:])
                        nc.scalar.copy(V[:, sc * 4 + j, :], pst[:])
                    # Q^T chunks (4 heads)
                    for dt in range(HPC):
                        psq = ps1.tile([128, SC], F32, tag=f"q{dt}")
                        for ht in range(NHT):
                            nc.tensor.matmul(
                                psq[:], wq_sb[:, ht, ts(dt, 128)], xts[ht][:],
                                start=(ht == 0), stop=(ht == NHT - 1),
                            )
                        nc.scalar.activation(
                            QT[:, dt, ts(sc, SC)], psq[:], Ident,
                            bias=bq_sb[:, dt : dt + 1],
                        )

            # ---------------- Stage 2: attention ----------------
            with (
                tc.tile_pool(name="wo", bufs=1) as wop,
                tc.tile_pool(name="es", bufs=8) as esp,
                tc.tile_pool(name="acc", bufs=2) as accp,
                tc.tile_pool(name="rs", bufs=2) as rsp,
            ):
                wo_sb = wop.tile([128, HPC, HID], BF16)
                nc.sync.dma_start(
                    wo_sb[:], wo.rearrange("(t p) d -> p t d", p=128)
                )

                with (
                    tc.tile_pool(name="psS", bufs=2, space="PSUM") as psS,
                    tc.tile_pool(name="psO", bufs=1, space="PSUM") as psO,
                    tc.tile_pool(name="psR", bufs=1, space="PSUM") as psR,
                    tc.tile_pool(name="psB", bufs=1, space="PSUM") as psB,
                ):
                  for qc in range(NSC):
                    nkt = 4 * qc + 4
                    psos = [
                        psO.tile([128, SC], F32, tag=f"o{h}", name=f"pso_{h}")
                        for h in range(HPC)
                    ]
                    accs = [
                        accp.tile([128, SC], F32, tag=f"a{h}", name=f"acc_{h}")
                        for h in range(HPC)
                    ]
                    for h in range(HPC):
                        nc.vector.memset(accs[h][:], 0.0)
                    for kt in range(nkt):
                        ess = []
                        for h in range(HPC):
                            ps = psS.tile([128, SC], F32, tag="s")
                            nc.tensor.matmul(
                                ps[:], KT[:, ts(kt, 128)], QT[:, h, ts(qc, SC)],
                                start=True, stop=True,
                            )
                            j = kt - 4 * qc
                            if j >= 0:
                                nc.vector.tensor_add(
                                    ps[:], ps[:], dmask_sb[:, ts(j, SC)]
                                )
                            es = esp.tile([128, SC], BF16, tag="es")
                            nc.scalar.activation(
                                es[:], ps[:], Exp,
                                bias=padb_sb[:, kt : kt + 1], scale=SCALE,
                            )
                            ess.append(es)
                        for h in range(HPC):
                            nc.tensor.matmul(
                                psos[h][:], V[:, kt, :], ess[h][:],
                                start=(kt == 0), stop=(kt == nkt - 1),
                            )
                            nc.vector.tensor_add(
                                accs[h][:], accs[h][:], ess[h][:]
                            )
                    for h in range(HPC):
                        psr_t = psR.tile([1, SC], F32, tag="r")
                        nc.tensor.matmul(
                            psr_t[:], ones_col[:], accs[h][:],
                            start=True, stop=True,
                        )
                        rs = rsp.tile([1, SC], F32, tag="rs")
                        nc.vector.reciprocal(rs[:], psr_t[:])
                        psb = psB.tile([128, SC], F32, tag="b")
                        nc.tensor.matmul(
                            psb[:], ones_row[:], rs[:], start=True, stop=True
                        )
                        bb = rsp.tile([128, SC], F32, tag="bb")
                        nc.scalar.copy(bb[:], psb[:])
                        nc.vector.tensor_mul(
                            OT[:, h, ts(qc, SC)], psos[h][:], bb[:]
                        )

                # ---------------- Stage 3: output projection ----------------
                with (
                    tc.tile_pool(name="outsb", bufs=4) as outp,
                    tc.tile_pool(name="ps3", bufs=1, space="PSUM") as ps3,
                ):
                    for st in range(NT):
                        pss = [
                            ps3.tile([128, SC], F32, tag=f"c{hc}", name=f"ps3_{hc}")
                            for hc in range(HID // SC)
                        ]
                        for dt in range(HPC):
                            for hc in range(HID // SC):
                                nc.tensor.matmul(
                                    pss[hc][:],
                                    OT[:, dt, ts(st, 128)],
                                    wo_sb[:, dt, ts(hc, SC)],
                                    start=(dt == 0), stop=(dt == HPC - 1),
                                )
                        for hc in range(HID // SC):
                            ot = outp.tile([128, SC], F32, tag="out")
                            nc.scalar.copy(ot[:], pss[hc][:])
                            nc.sync.dma_start(
                                out[ts(st, 128), ts(hc, SC)], ot[:]
                            )
    nc.compile()
    return nc


def _get_program():
    global _PROGRAM
    if _PROGRAM is None:
        _PROGRAM = _build_program()
    return _PROGRAM


def kernel(**inputs):
    global LAST_RESULT
    hs = np.ascontiguousarray(inputs["hidden_states"], dtype=np.float32)
    pad = np.ascontiguousarray(inputs["padding_mask"], dtype=np.float32)
    Wq = np.asarray(inputs["Wq"], dtype=np.float32)
    Wk = np.asarray(inputs["Wk"], dtype=np.float32)
    Wv = np.asarray(inputs["Wv"], dtype=np.float32)
    Wo = np.asarray(inputs["Wo"], dtype=np.float32)
    bq_v = np.asarray(inputs["bq"], dtype=np.float32)
    bk_v = np.asarray(inputs["bk"], dtype=np.float32)
    bv_v = np.asarray(inputs["bv"], dtype=np.float32)
    bo_v = np.asarray(inputs["bo"], dtype=np.float32)

    xTs = [np.ascontiguousarray(hs[b].T).astype(NP_BF16) for b in range(B)]
    WqT = Wq.T  # [HID, HID]
    WkT = np.ascontiguousarray(Wk.T).astype(NP_BF16)  # [HID, D]
    WvT = np.ascontiguousarray(Wv.T).astype(NP_BF16)
    WoT = Wo.T  # [HID, HID]

    # 4 diagonal-block masks in S^T layout: mask_j[p, f] = 0 if p+128*j <= f
    p_i = np.arange(128)[:, None]
    f_i = np.arange(SC)[None, :]
    dmask = np.empty((128, 4 * SC), np.float32)
    for j in range(4):
        dmask[:, j * SC : (j + 1) * SC] = np.where(
            p_i + 128 * j <= f_i, 0.0, NEG
        ).astype(np.float32)

    padbs = [
        np.ascontiguousarray((NEG * pad[b]).reshape(NT, 128).T) for b in range(B)
    ]
    bqs = [
        np.ascontiguousarray(
            bq_v[hg * DPH : (hg + 1) * DPH].reshape(HPC, 128).T
        )
        for hg in range(HPC)
    ]
    bkv = np.ascontiguousarray(np.stack([bk_v, bv_v], axis=1))  # [128, 2]

    nc = _get_program()
    in_maps = []
    for c in range(NCORES):
        b, hg = c // 4, c % 4
        in_maps.append(
            {
                "xT": xTs[b],
                "wq": np.ascontiguousarray(
                    WqT[:, hg * DPH : (hg + 1) * DPH]
                ).astype(NP_BF16),
                "wk": WkT,
                "wv": WvT,
                "wo": np.ascontiguousarray(
                    WoT[hg * DPH : (hg + 1) * DPH, :]
                ).astype(NP_BF16),
                "bq": bqs[hg],
                "bkv": bkv,
                "padb": padbs[b],
                "dmask": dmask,
            }
        )

    LAST_RESULT = run_bass_kernel_spmd(nc, in_maps, list(range(NCORES)))
    res = LAST_RESULT.results

    outp = np.zeros((B, S, HID), np.float32)
    for c in range(NCORES):
        outp[c // 4] += res[c]["out"]
    outp += bo_v[None, None, :]
    return outp


if __name__ == "__main__":
    rng = np.random.default_rng(0)
    demo = {
        "hidden_states": rng.standard_normal((B, S, HID), dtype=np.float32),
        "causal_mask": np.triu(np.ones((1, 1, S, S), np.float32), k=1),
        "padding_mask": np.zeros((B, S), np.float32),
        "Wq": (rng.standard_normal((HID, HID), dtype=np.float32) * 0.02),
        "bq": np.zeros((HID,), np.float32),
        "Wk": (rng.standard_normal((D, HID), dtype=np.float32) * 0.02),
        "bk": np.zeros((D,), np.float32),
        "Wv": (rng.standard_normal((D, HID), dtype=np.float32) * 0.02),
        "bv": np.zeros((D,), np.float32),
        "Wo": (rng.standard_normal((HID, HID), dtype=np.float32) * 0.02),
        "bo": np.zeros((HID,), np.float32),
    }
    o = kernel(**demo)
    print("kernel output", o.shape, o.dtype, float(np.abs(o).mean()))



# revision 2
# speedup vs baseline: 1.5383x; 1.5383x over previous
"""MQA (GQA, 1 KV group) attention kernel for 8 Trainium2 NeuronCores.

Sharding: core c -> batch b = c//4, head-group hg = c%4 (4 of 16 query heads).
Each core computes Q/K/V projections from x[b]^T, causal attention for its 4
heads in transposed layout, and a partial output projection.  Host sums the 4
partials per batch and adds bo.

Schedule is built to keep the PE streaming at its max p-state:
 - attention q-chunks of 128 rows; per kv-tile ONE 4-head-wide scores matmul
   [128kv x 512(h,q)], ONE exp activation, ONE AV matmul, and ONE fused
   rowsum+broadcast matmul (ones^T @ es accumulated in PSUM) -- 3 PE + 1 ACT
   instructions per kv tile, all 512 free columns.
 - causal diag handled by a multiplicative 0/1 bf16 mask on DVE; padding mask
   enters as a per-kv-partition bias in the exp activation.
 - normalization: reciprocal_approx_fast + one DVE mul per chunk.
 - stage-1 bias adds and stage-3 PSUM drains on DVE; exp one-kt-ahead
   software pipeline; outproj(qc-1) interleaved after attn(qc) so the PE has
   filler work while the scalar engine catches up.
"""

import sys

sys.path.insert(0, "/opt/trn_rl_repo")

import ml_dtypes
import numpy as np

import concourse.bass as bass
import concourse.tile as tile
from concourse import bacc
from concourse import mybir
from concourse.bass import ts
from concourse.bass_utils import run_bass_kernel_spmd
from concourse.masks import make_identity

B, S, HID = 2, 2048, 2048
H, D = 16, 128
HPC = 4              # heads per core
DPH = HPC * D        # 512
NCORES = 8
SC1 = 512            # stage-1 s-chunk
NSC1 = S // SC1      # 4
QC = 128             # attention q-chunk
NQC = S // QC        # 16
NT = S // 128        # 16
NHT = HID // 128     # 16
SCALE = 1.0 / float(np.sqrt(D))
NEG = -1.0e9

F32 = mybir.dt.float32
BF16 = mybir.dt.bfloat16
NP_BF16 = ml_dtypes.bfloat16

_PROGRAM = None
LAST_RESULT = None


def _build_program():
    nc = bacc.Bacc()
    xT = nc.declare_dram_parameter("xT", [NT, 128, S], BF16, isOutput=False)
    wq = nc.declare_dram_parameter("wq", [HID, DPH], BF16, isOutput=False)
    wk = nc.declare_dram_parameter("wk", [HID, D], BF16, isOutput=False)
    wv = nc.declare_dram_parameter("wv", [HID, D], BF16, isOutput=False)
    wo = nc.declare_dram_parameter("wo", [DPH, HID], BF16, isOutput=False)
    bq = nc.declare_dram_parameter("bq", [128, HPC], F32, isOutput=False)
    bkv = nc.declare_dram_parameter("bkv", [128, 2], F32, isOutput=False)
    padb = nc.declare_dram_parameter("padb", [128, NT], F32, isOutput=False)
    mask4 = nc.declare_dram_parameter("mask4", [128, HPC, QC], BF16, isOutput=False)
    out = nc.declare_dram_parameter("out", [S, HID], F32, isOutput=True)

    Exp = mybir.ActivationFunctionType.Exp

    with tile.TileContext(nc) as tc:
        with (
            tc.tile_pool(name="consts", bufs=1) as consts,
            tc.tile_pool(name="persist", bufs=1) as persist,
            tc.tile_pool(name="esb", bufs=1) as esb,
            tc.tile_pool(name="ps", bufs=1, space="PSUM") as ps,
        ):
            # ---- constants / small params (front of DMA queue) ----
            ident = consts.tile([128, 128], BF16)
            make_identity(nc, ident[:])
            ones128 = consts.tile([128, 128], BF16)
            nc.vector.memset(ones128[:], 1.0)
            bq_sb = consts.tile([128, HPC], F32)
            nc.sync.dma_start(bq_sb[:], bq[:])
            bkv_sb = consts.tile([128, 2], F32)
            nc.sync.dma_start(bkv_sb[:], bkv[:])
            padb_sb = consts.tile([128, NT], F32)
            nc.sync.dma_start(padb_sb[:], padb[:])
            mask_sb = consts.tile([128, HPC, QC], BF16)
            nc.sync.dma_start(mask_sb[:], mask4[:])
            wk_sb = consts.tile([128, NHT, D], BF16)
            nc.sync.dma_start(wk_sb[:], wk.rearrange("(t p) d -> p t d", p=128))
            wv_sb = consts.tile([128, NHT, D], BF16)
            nc.sync.dma_start(wv_sb[:], wv.rearrange("(t p) d -> p t d", p=128))

            # ---- big inputs: x^T tiles streamed chunk-major ----
            xts = persist.tile([128, NHT, S], BF16)
            for ht in range(NHT):
                nc.sync.dma_start(
                    xts[:, ht, ts(0, SC1)], xT[ht, :, ts(0, SC1)]
                )
            wq_sb = persist.tile([128, NHT, DPH], BF16)
            nc.sync.dma_start(wq_sb[:], wq.rearrange("(t p) d -> p t d", p=128))
            for sc in range(1, NSC1):
                for ht in range(NHT):
                    nc.sync.dma_start(
                        xts[:, ht, ts(sc, SC1)], xT[ht, :, ts(sc, SC1)]
                    )
            wo_sb = persist.tile([128, HPC, HID], BF16)
            nc.sync.dma_start(wo_sb[:], wo.rearrange("(t p) d -> p t d", p=128))

            # ---- persistent activations ----
            KT = persist.tile([128, S], BF16)         # K^T [d, kv]
            V = persist.tile([128, NT, 128], BF16)    # V tiles [kv_p, kt, d]
            QT = persist.tile([128, HPC, S], BF16)    # Q^T [d, h, q]
            OT = persist.tile([128, HPC, S], BF16)    # normalized (exp S)V ^T

            def stage1(sc):
                # K chunk
                psk = ps.tile([128, SC1], F32, tag="bg", bufs=2, name="psk")
                for ht in range(NHT):
                    nc.tensor.matmul(
                        psk[:], wk_sb[:, ht, :], xts[:, ht, ts(sc, SC1)],
                        start=(ht == 0), stop=(ht == NHT - 1),
                    )
                nc.vector.tensor_scalar_add(
                    KT[:, ts(sc, SC1)], psk[:], bkv_sb[:, 0:1]
                )
                # V chunk -> transpose into V tiles
                psv = ps.tile([128, SC1], F32, tag="bg", bufs=2, name="psv")
                for ht in range(NHT):
                    nc.tensor.matmul(
                        psv[:], wv_sb[:, ht, :], xts[:, ht, ts(sc, SC1)],
                        start=(ht == 0), stop=(ht == NHT - 1),
                    )
                vt_s = esb.tile([128, SC1], BF16, tag="vt", bufs=2, name="vt")
                nc.vector.tensor_scalar_add(vt_s[:], psv[:], bkv_sb[:, 1:2])
                # Q head 0
                psq = ps.tile([128, SC1], F32, tag="bg", bufs=2, name="psq0")
                for ht in range(NHT):
                    nc.tensor.matmul(
                        psq[:], wq_sb[:, ht, ts(0, 128)], xts[:, ht, ts(sc, SC1)],
                        start=(ht == 0), stop=(ht == NHT - 1),
                    )
                nc.vector.tensor_scalar_add(
                    QT[:, 0, ts(sc, SC1)], psq[:], bq_sb[:, 0:1]
                )
                # transposes of V chunk (4 tiles into one PSUM slot)
                pstr = ps.tile([128, 4, 128], BF16, tag="bg", bufs=2, name="pstr")
                for j in range(4):
                    nc.tensor.transpose(
                        pstr[:, j, :], vt_s[:, ts(j, 128)], ident[:]
                    )
                nc.scalar.copy(V[:, 4 * sc : 4 * sc + 4, :], pstr[:])
                # Q heads 1..3
                for dt in range(1, HPC):
                    psq = ps.tile(
                        [128, SC1], F32, tag="bg", bufs=2, name=f"psq{dt}"
                    )
                    for ht in range(NHT):
                        nc.tensor.matmul(
                            psq[:], wq_sb[:, ht, ts(dt, 128)],
                            xts[:, ht, ts(sc, SC1)],
                            start=(ht == 0), stop=(ht == NHT - 1),
                        )
                    nc.vector.tensor_scalar_add(
                        QT[:, dt, ts(sc, SC1)], psq[:], bq_sb[:, dt : dt + 1]
                    )

            def outproj(qc):
                # out rows [qc*128, (qc+1)*128), all HID cols
                for hc in range(HID // SC1):
                    ps3 = ps.tile(
                        [128, SC1], F32, tag="bg", bufs=2, name=f"ps3_{hc}"
                    )
                    for dt in range(HPC):
                        nc.tensor.matmul(
                            ps3[:],
                            OT[:, dt, ts(qc, QC)],
                            wo_sb[:, dt, ts(hc, SC1)],
                            start=(dt == 0), stop=(dt == HPC - 1),
                        )
                    ot = esb.tile([128, SC1], F32, tag="out", bufs=3, name="ot")
                    nc.vector.tensor_scalar_add(ot[:], ps3[:], 0.0)
                    nc.sync.dma_start(out[ts(qc, QC), ts(hc, SC1)], ot[:])

            def attn(qc, interleave=None):
                nkt = qc + 1
                pso = ps.tile([128, HPC, QC], F32, tag="o", bufs=2, name="pso")
                psr = ps.tile([128, HPC, QC], F32, tag="r", bufs=1, name="psr")
                pend = []

                def consume():
                    es, kt = pend.pop(0)
                    nc.tensor.matmul(
                        pso[:], V[:, kt, :], es[:],
                        start=(kt == 0), stop=(kt == nkt - 1),
                    )
                    nc.tensor.matmul(
                        psr[:], ones128[:], es[:],
                        start=(kt == 0), stop=(kt == nkt - 1),
                    )

                for kt in range(nkt):
                    psS = ps.tile(
                        [128, HPC, QC], F32, tag="s", bufs=3, name="psS"
                    )
                    nc.tensor.matmul(
                        psS[:], KT[:, ts(kt, 128)], QT[:, :, ts(qc, QC)],
                        start=True, stop=True,
                    )
                    es = esb.tile([128, HPC, QC], BF16, tag="es", bufs=4, name="es")
                    nc.scalar.activation(
                        es[:], psS[:], Exp,
                        bias=padb_sb[:, kt : kt + 1], scale=SCALE,
                    )
                    if kt == qc:
                        nc.vector.tensor_mul(es[:], es[:], mask_sb[:])
                    if pend:
                        consume()
                    pend.append((es, kt))
                # filler PE work while the last exp drains
                if interleave is not None:
                    interleave()
                while pend:
                    consume()
                # normalize: OT[:, h, qc block] = pso * 1/psr
                rec = esb.tile([128, HPC, QC], F32, tag="rec", bufs=2, name="rec")
                nc.vector.reciprocal_approx_fast(rec[:], psr[:])
                nc.vector.tensor_mul(OT[:, :, ts(qc, QC)], pso[:], rec[:])

            # ---------------- main schedule ----------------
            for sc in range(NSC1):
                stage1(sc)
                for qc in range(4 * sc, 4 * sc + 4):
                    attn(
                        qc,
                        interleave=(
                            (lambda q=qc: outproj(q - 1)) if qc >= 1 else None
                        ),
                    )
            outproj(NQC - 1)

    nc.compile()
    return nc


def _get_program():
    global _PROGRAM
    if _PROGRAM is None:
        _PROGRAM = _build_program()
    return _PROGRAM


def kernel(**inputs):
    global LAST_RESULT
    hs = np.ascontiguousarray(inputs["hidden_states"], dtype=np.float32)
    pad = np.ascontiguousarray(inputs["padding_mask"], dtype=np.float32)
    Wq = np.asarray(inputs["Wq"], dtype=np.float32)
    Wk = np.asarray(inputs["Wk"], dtype=np.float32)
    Wv = np.asarray(inputs["Wv"], dtype=np.float32)
    Wo = np.asarray(inputs["Wo"], dtype=np.float32)
    bq_v = np.asarray(inputs["bq"], dtype=np.float32)
    bk_v = np.asarray(inputs["bk"], dtype=np.float32)
    bv_v = np.asarray(inputs["bv"], dtype=np.float32)
    bo_v = np.asarray(inputs["bo"], dtype=np.float32)

    xTs = [
        np.ascontiguousarray(hs[b].T).astype(NP_BF16).reshape(NT, 128, S)
        for b in range(B)
    ]
    WqT = Wq.T  # [HID, HID]
    WkT = np.ascontiguousarray(Wk.T).astype(NP_BF16)  # [HID, D]
    WvT = np.ascontiguousarray(Wv.T).astype(NP_BF16)
    WoT = Wo.T  # [HID, HID]

    # causal 0/1 mask for the diagonal tile, [128 kv, h, 128 q]
    p_i = np.arange(128)[:, None]
    q_i = np.arange(QC)[None, :]
    m = (q_i >= p_i).astype(np.float32)
    mask4 = np.ascontiguousarray(
        np.broadcast_to(m[:, None, :], (128, HPC, QC))
    ).astype(NP_BF16)

    padbs = [
        np.ascontiguousarray((NEG * pad[b]).reshape(NT, 128).T) for b in range(B)
    ]
    bqs = [
        np.ascontiguousarray(
            bq_v[hg * DPH : (hg + 1) * DPH].reshape(HPC, 128).T
        )
        for hg in range(HPC)
    ]
    bkv = np.ascontiguousarray(np.stack([bk_v, bv_v], axis=1))  # [128, 2]

    nc = _get_program()
    in_maps = []
    for c in range(NCORES):
        b, hg = c // 4, c % 4
        in_maps.append(
            {
                "xT": xTs[b],
                "wq": np.ascontiguousarray(
                    WqT[:, hg * DPH : (hg + 1) * DPH]
                ).astype(NP_BF16),
                "wk": WkT,
                "wv": WvT,
                "wo": np.ascontiguousarray(
                    WoT[hg * DPH : (hg + 1) * DPH, :]
                ).astype(NP_BF16),
                "bq": bqs[hg],
                "bkv": bkv,
                "padb": padbs[b],
                "mask4": mask4,
            }
        )

    LAST_RESULT = run_bass_kernel_spmd(nc, in_maps, list(range(NCORES)))
    res = LAST_RESULT.results

    outp = np.zeros((B, S, HID), np.float32)
    for c in range(NCORES):
        outp[c // 4] += res[c]["out"]
    outp += bo_v[None, None, :]
    return outp


if __name__ == "__main__":
    rng = np.random.default_rng(0)
    demo = {
        "hidden_states": rng.standard_normal((B, S, HID), dtype=np.float32),
        "causal_mask": np.triu(np.ones((1, 1, S, S), np.float32), k=1),
        "padding_mask": np.zeros((B, S), np.float32),
        "Wq": (rng.standard_normal((HID, HID), dtype=np.float32) * 0.02),
        "bq": np.zeros((HID,), np.float32),
        "Wk": (rng.standard_normal((D, HID), dtype=np.float32) * 0.02),
        "bk": np.zeros((D,), np.float32),
        "Wv": (rng.standard_normal((D, HID), dtype=np.float32) * 0.02),
        "bv": np.zeros((D,), np.float32),
        "Wo": (rng.standard_normal((HID, HID), dtype=np.float32) * 0.02),
        "bo": np.zeros((HID,), np.float32),
    }
    o = kernel(**demo)
    print("kernel output", o.shape, o.dtype, float(np.abs(o).mean()))


# revision 7
# speedup vs baseline: 1.6412x; 1.0669x over previous
"""MQA (GQA, 1 KV group) attention kernel for 8 Trainium2 NeuronCores.

Sharding: core c -> batch b = c//4, head-group hg = c%4 (4 of 16 query heads).
Each core computes Q/K/V projections from x[b]^T, causal attention for its 4
heads in transposed layout, and a partial output projection.  Host sums the 4
partials per batch and adds bo.

Schedule is built to keep the PE streaming at its max p-state:
 - attention q-chunks of 128 rows; per kv-tile ONE 4-head-wide scores matmul
   [128kv x 512(h,q)], ONE exp activation, ONE AV matmul, and ONE fused
   rowsum+broadcast matmul (ones^T @ es accumulated in PSUM) -- 3 PE + 1 ACT
   instructions per kv tile, all 512 free columns.
 - causal diag handled by a multiplicative 0/1 bf16 mask on DVE; padding mask
   enters as a per-kv-partition bias in the exp activation.
 - normalization: reciprocal_approx_fast + one DVE mul per chunk.
 - stage-1 bias adds and stage-3 PSUM drains on DVE; exp one-kt-ahead
   software pipeline; outproj(qc-1) interleaved after attn(qc) so the PE has
   filler work while the scalar engine catches up.
"""

import sys

sys.path.insert(0, "/opt/trn_rl_repo")

import ml_dtypes
import numpy as np

import concourse.bass as bass
import concourse.tile as tile
from concourse import bacc
from concourse import mybir
from concourse.bass import ts
from concourse.bass_utils import run_bass_kernel_spmd
from concourse.masks import make_identity

B, S, HID = 2, 2048, 2048
H, D = 16, 128
HPC = 4              # heads per core
DPH = HPC * D        # 512
NCORES = 8
SC1 = 512            # stage-1 s-chunk
NSC1 = S // SC1      # 4
QC = 128             # attention q-chunk
NQC = S // QC        # 16
NT = S // 128        # 16
NHT = HID // 128     # 16
SCALE = 1.0 / float(np.sqrt(D))
NEG = -1.0e9

F32 = mybir.dt.float32
BF16 = mybir.dt.bfloat16
NP_BF16 = ml_dtypes.bfloat16

_PROGRAM = None
LAST_RESULT = None


def _build_program():
    nc = bacc.Bacc()
    # all big inputs pre-shuffled on host so each DMA reads long contiguous
    # per-partition lines (16KB) instead of 1KB strided rows
    xT = nc.declare_dram_parameter("xT", [128, NSC1, NHT, SC1], BF16, isOutput=False)
    wq = nc.declare_dram_parameter("wq", [128, NHT, DPH], BF16, isOutput=False)
    wk = nc.declare_dram_parameter("wk", [128, NHT, D], BF16, isOutput=False)
    wv = nc.declare_dram_parameter("wv", [128, NHT, D], BF16, isOutput=False)
    wo = nc.declare_dram_parameter("wo", [128, HPC, HID], BF16, isOutput=False)
    bq = nc.declare_dram_parameter("bq", [128, HPC], F32, isOutput=False)
    bkv = nc.declare_dram_parameter("bkv", [128, 2], F32, isOutput=False)
    padb = nc.declare_dram_parameter("padb", [128, NT], F32, isOutput=False)
    mask4 = nc.declare_dram_parameter("mask4", [128, HPC, QC], BF16, isOutput=False)
    out = nc.declare_dram_parameter("out", [S, HID], F32, isOutput=True)

    Exp = mybir.ActivationFunctionType.Exp

    with tile.TileContext(nc) as tc:
        with (
            tc.tile_pool(name="consts", bufs=1) as consts,
            tc.tile_pool(name="persist", bufs=1) as persist,
            tc.tile_pool(name="esb", bufs=1) as esb,
            tc.tile_pool(name="ps", bufs=1, space="PSUM") as ps,
        ):
            # ---- constants / small params (front of DMA queue) ----
            ident = consts.tile([128, 128], BF16)
            make_identity(nc, ident[:])
            ones128 = consts.tile([128, 128], BF16)
            nc.vector.memset(ones128[:], 1.0)
            bq_sb = consts.tile([128, HPC], F32)
            nc.sync.dma_start(bq_sb[:], bq[:])
            bkv_sb = consts.tile([128, 2], F32)
            nc.sync.dma_start(bkv_sb[:], bkv[:])
            padb_sb = consts.tile([128, NT], F32)
            nc.sync.dma_start(padb_sb[:], padb[:])
            mask_sb = consts.tile([128, HPC, QC], BF16)
            nc.sync.dma_start(mask_sb[:], mask4[:])
            wk_sb = consts.tile([128, NHT, D], BF16)
            nc.sync.dma_start(wk_sb[:], wk[:])
            wv_sb = consts.tile([128, NHT, D], BF16)
            nc.sync.dma_start(wv_sb[:], wv[:])

            # ---- big inputs, chunk-major, contiguous 16KB lines ----
            xts = persist.tile([128, NSC1, NHT, SC1], BF16)
            nc.sync.dma_start(xts[:, 0], xT[:, 0])
            wq_sb = persist.tile([128, NHT, DPH], BF16)
            nc.sync.dma_start(wq_sb[:, 0:8], wq[:, 0:8])
            nc.sync.dma_start(wq_sb[:, 8:16], wq[:, 8:16])
            wo_sb = persist.tile([128, HPC, HID], BF16)
            nc.sync.dma_start(wo_sb[:], wo[:])
            for sc in range(1, NSC1):
                nc.sync.dma_start(xts[:, sc], xT[:, sc])

            # ---- persistent activations ----
            KT = persist.tile([128, S], BF16)         # K^T [d, kv]
            V = persist.tile([128, NT, 128], BF16)    # V tiles [kv_p, kt, d]
            QT = persist.tile([128, HPC, S], BF16)    # Q^T [d, h, q]
            OT = persist.tile([128, HPC, S], BF16)    # normalized (exp S)V ^T

            def stage1(sc):
                # K chunk
                psk = ps.tile([128, SC1], F32, tag="bg", bufs=2, name="psk")
                for ht in range(NHT):
                    nc.tensor.matmul(
                        psk[:], wk_sb[:, ht, :], xts[:, sc, ht, :],
                        start=(ht == 0), stop=(ht == NHT - 1),
                    )
                nc.vector.tensor_scalar_add(
                    KT[:, ts(sc, SC1)], psk[:], bkv_sb[:, 0:1]
                )
                # V chunk
                psv = ps.tile([128, SC1], F32, tag="bg", bufs=2, name="psv")
                for ht in range(NHT):
                    nc.tensor.matmul(
                        psv[:], wv_sb[:, ht, :], xts[:, sc, ht, :],
                        start=(ht == 0), stop=(ht == NHT - 1),
                    )
                vt_s = esb.tile([128, SC1], BF16, tag="vt", bufs=2, name="vt")
                nc.vector.tensor_scalar_add(vt_s[:], psv[:], bkv_sb[:, 1:2])
                # Q heads
                for dt in range(HPC):
                    psq = ps.tile(
                        [128, SC1], F32, tag="bg", bufs=2, name=f"psq{dt}"
                    )
                    for ht in range(NHT):
                        nc.tensor.matmul(
                            psq[:], wq_sb[:, ht, ts(dt, 128)],
                            xts[:, sc, ht, :],
                            start=(ht == 0), stop=(ht == NHT - 1),
                        )
                    nc.vector.tensor_scalar_add(
                        QT[:, dt, ts(sc, SC1)], psq[:], bq_sb[:, dt : dt + 1]
                    )
                # V transposes last: vt_s is long since drained by now, so
                # these never block the in-order PE queue
                pstr = ps.tile([128, 4, 128], BF16, tag="bg", bufs=2, name="pstr")
                for j in range(4):
                    nc.tensor.transpose(
                        pstr[:, j, :], vt_s[:, ts(j, 128)], ident[:]
                    )
                nc.scalar.copy(V[:, 4 * sc : 4 * sc + 4, :], pstr[:])

            def outproj(qc):
                # out rows [qc*128, (qc+1)*128), all HID cols
                for hc in range(HID // SC1):
                    ps3 = ps.tile(
                        [128, SC1], F32, tag="bg", bufs=2, name=f"ps3_{hc}"
                    )
                    for dt in range(HPC):
                        nc.tensor.matmul(
                            ps3[:],
                            OT[:, dt, ts(qc, QC)],
                            wo_sb[:, dt, ts(hc, SC1)],
                            start=(dt == 0), stop=(dt == HPC - 1),
                        )
                    ot = esb.tile([128, SC1], F32, tag="out", bufs=3, name="ot")
                    nc.vector.tensor_scalar_add(ot[:], ps3[:], 0.0)
                    nc.sync.dma_start(out[ts(qc, QC), ts(hc, SC1)], ot[:])

            def attn(qc, interleave=None):
                nkt = qc + 1
                pso = ps.tile([128, HPC, QC], F32, tag="o", bufs=2, name="pso")
                psr = ps.tile([128, HPC, QC], F32, tag="r", bufs=1, name="psr")
                pend = []

                def consume():
                    es, kt = pend.pop(0)
                    nc.tensor.matmul(
                        pso[:], V[:, kt, :], es[:],
                        start=(kt == 0), stop=(kt == nkt - 1),
                    )
                    nc.tensor.matmul(
                        psr[:], ones128[:], es[:],
                        start=(kt == 0), stop=(kt == nkt - 1),
                    )

                for kt in range(nkt):
                    psS = ps.tile(
                        [128, HPC, QC], F32, tag="s", bufs=3, name="psS"
                    )
                    nc.tensor.matmul(
                        psS[:], KT[:, ts(kt, 128)], QT[:, :, ts(qc, QC)],
                        start=True, stop=True,
                    )
                    es = esb.tile([128, HPC, QC], BF16, tag="es", bufs=4, name="es")
                    nc.scalar.activation(
                        es[:], psS[:], Exp,
                        bias=padb_sb[:, kt : kt + 1], scale=SCALE,
                    )
                    if kt == qc:
                        nc.vector.tensor_mul(es[:], es[:], mask_sb[:])
                    if pend:
                        consume()
                    pend.append((es, kt))
                # filler PE work while the last exp drains
                if interleave is not None:
                    interleave()
                while pend:
                    consume()
                # normalize: OT[:, h, qc block] = pso * 1/psr
                rec = esb.tile([128, HPC, QC], F32, tag="rec", bufs=2, name="rec")
                nc.vector.reciprocal_approx_fast(rec[:], psr[:])
                nc.vector.tensor_mul(OT[:, :, ts(qc, QC)], pso[:], rec[:])

            # ---------------- main schedule ----------------
            for sc in range(NSC1):
                stage1(sc)
                for qc in range(4 * sc, 4 * sc + 4):
                    attn(
                        qc,
                        interleave=(
                            (lambda q=qc: outproj(q - 1)) if qc >= 1 else None
                        ),
                    )
            outproj(NQC - 1)

    nc.compile()
    return nc


def _get_program():
    global _PROGRAM
    if _PROGRAM is None:
        _PROGRAM = _build_program()
    return _PROGRAM


def kernel(**inputs):
    global LAST_RESULT
    hs = np.ascontiguousarray(inputs["hidden_states"], dtype=np.float32)
    pad = np.ascontiguousarray(inputs["padding_mask"], dtype=np.float32)
    Wq = np.asarray(inputs["Wq"], dtype=np.float32)
    Wk = np.asarray(inputs["Wk"], dtype=np.float32)
    Wv = np.asarray(inputs["Wv"], dtype=np.float32)
    Wo = np.asarray(inputs["Wo"], dtype=np.float32)
    bq_v = np.asarray(inputs["bq"], dtype=np.float32)
    bk_v = np.asarray(inputs["bk"], dtype=np.float32)
    bv_v = np.asarray(inputs["bv"], dtype=np.float32)
    bo_v = np.asarray(inputs["bo"], dtype=np.float32)

    # x^T pre-shuffled to [p, sc, ht, c]: partition lines are 16KB contiguous
    xTs = [
        np.ascontiguousarray(
            hs[b].T.reshape(NHT, 128, NSC1, SC1).transpose(1, 2, 0, 3)
        ).astype(NP_BF16)
        for b in range(B)
    ]
    WqT = Wq.T  # [HID, HID]
    # [p, ht, d] shuffles
    WkT = np.ascontiguousarray(
        Wk.T.reshape(NHT, 128, D).transpose(1, 0, 2)
    ).astype(NP_BF16)
    WvT = np.ascontiguousarray(
        Wv.T.reshape(NHT, 128, D).transpose(1, 0, 2)
    ).astype(NP_BF16)
    WoT = Wo.T  # [HID, HID]

    # causal 0/1 mask for the diagonal tile, [128 kv, h, 128 q]
    p_i = np.arange(128)[:, None]
    q_i = np.arange(QC)[None, :]
    m = (q_i >= p_i).astype(np.float32)
    mask4 = np.ascontiguousarray(
        np.broadcast_to(m[:, None, :], (128, HPC, QC))
    ).astype(NP_BF16)

    padbs = [
        np.ascontiguousarray((NEG * pad[b]).reshape(NT, 128).T) for b in range(B)
    ]
    bqs = [
        np.ascontiguousarray(
            bq_v[hg * DPH : (hg + 1) * DPH].reshape(HPC, 128).T
        )
        for hg in range(HPC)
    ]
    bkv = np.ascontiguousarray(np.stack([bk_v, bv_v], axis=1))  # [128, 2]

    nc = _get_program()
    in_maps = []
    for c in range(NCORES):
        b, hg = c // 4, c % 4
        in_maps.append(
            {
                "xT": xTs[b],
                "wq": np.ascontiguousarray(
                    WqT[:, hg * DPH : (hg + 1) * DPH]
                    .reshape(NHT, 128, DPH)
                    .transpose(1, 0, 2)
                ).astype(NP_BF16),
                "wk": WkT,
                "wv": WvT,
                "wo": np.ascontiguousarray(
                    WoT[hg * DPH : (hg + 1) * DPH, :]
                    .reshape(HPC, 128, HID)
                    .transpose(1, 0, 2)
                ).astype(NP_BF16),
                "bq": bqs[hg],
                "bkv": bkv,
                "padb": padbs[b],
                "mask4": mask4,
            }
        )

    LAST_RESULT = run_bass_kernel_spmd(nc, in_maps, list(range(NCORES)))
    res = LAST_RESULT.results

    outp = np.zeros((B, S, HID), np.float32)
    for c in range(NCORES):
        outp[c // 4] += res[c]["out"]
    outp += bo_v[None, None, :]
    return outp


if __name__ == "__main__":
    rng = np.random.default_rng(0)
    demo = {
        "hidden_states": rng.standard_normal((B, S, HID), dtype=np.float32),
        "causal_mask": np.triu(np.ones((1, 1, S, S), np.float32), k=1),
        "padding_mask": np.zeros((B, S), np.float32),
        "Wq": (rng.standard_normal((HID, HID), dtype=np.float32) * 0.02),
        "bq": np.zeros((HID,), np.float32),
        "Wk": (rng.standard_normal((D, HID), dtype=np.float32) * 0.02),
        "bk": np.zeros((D,), np.float32),
        "Wv": (rng.standard_normal((D, HID), dtype=np.float32) * 0.02),
        "bv": np.zeros((D,), np.float32),
        "Wo": (rng.standard_normal((HID, HID), dtype=np.float32) * 0.02),
        "bo": np.zeros((HID,), np.float32),
    }
    o = kernel(**demo)
    print("kernel output", o.shape, o.dtype, float(np.abs(o).mean()))


# revision 11
# speedup vs baseline: 1.6834x; 1.0257x over previous
"""MQA (GQA, 1 KV group) attention kernel for 8 Trainium2 NeuronCores.

Sharding: core c -> batch b = c//4, head-group hg = c%4 (4 of 16 query heads).
Each core computes Q/K/V projections from x[b]^T, causal attention for its 4
heads in transposed layout, and a partial output projection.  Host sums the 4
partials per batch and adds bo.

Schedule is built to keep the PE streaming at its max p-state:
 - attention q-chunks of 128 rows; per kv-tile ONE 4-head-wide scores matmul
   [128kv x 512(h,q)], ONE exp activation, ONE AV matmul, and ONE fused
   rowsum+broadcast matmul (ones^T @ es accumulated in PSUM) -- 3 PE + 1 ACT
   instructions per kv tile, all 512 free columns.
 - causal diag handled by a multiplicative 0/1 bf16 mask on DVE; padding mask
   enters as a per-kv-partition bias in the exp activation.
 - normalization: reciprocal_approx_fast + one DVE mul per chunk.
 - stage-1 bias adds and stage-3 PSUM drains on DVE; exp one-kt-ahead
   software pipeline; outproj(qc-1) interleaved after attn(qc) so the PE has
   filler work while the scalar engine catches up.
"""

import sys

sys.path.insert(0, "/opt/trn_rl_repo")

import ml_dtypes
import numpy as np

import concourse.bass as bass
import concourse.tile as tile
from concourse import bacc
from concourse import mybir
from concourse.bass import ts
from concourse.bass_utils import run_bass_kernel_spmd
from concourse.masks import make_identity

B, S, HID = 2, 2048, 2048
H, D = 16, 128
HPC = 4              # heads per core
DPH = HPC * D        # 512
NCORES = 8
SC1 = 512            # stage-1 s-chunk
NSC1 = S // SC1      # 4
QC = 128             # attention q-chunk
NQC = S // QC        # 16
NT = S // 128        # 16
NHT = HID // 128     # 16
SCALE = 1.0 / float(np.sqrt(D))
NEG = -1.0e9

F32 = mybir.dt.float32
BF16 = mybir.dt.bfloat16
NP_BF16 = ml_dtypes.bfloat16

_PROGRAM = None
LAST_RESULT = None


def _build_program():
    nc = bacc.Bacc()
    # all big inputs pre-shuffled on host so each DMA reads long contiguous
    # per-partition lines (16KB) instead of 1KB strided rows
    xT = nc.declare_dram_parameter("xT", [128, NSC1, NHT, SC1], BF16, isOutput=False)
    wq = nc.declare_dram_parameter("wq", [128, NHT, DPH], BF16, isOutput=False)
    wk = nc.declare_dram_parameter("wk", [128, NHT, D], BF16, isOutput=False)
    wv = nc.declare_dram_parameter("wv", [128, NHT, D], BF16, isOutput=False)
    wo = nc.declare_dram_parameter("wo", [128, HPC, HID], BF16, isOutput=False)
    bq = nc.declare_dram_parameter("bq", [128, HPC], F32, isOutput=False)
    bkv = nc.declare_dram_parameter("bkv", [128, 2], F32, isOutput=False)
    padb = nc.declare_dram_parameter("padb", [128, NT], F32, isOutput=False)
    mask4 = nc.declare_dram_parameter("mask4", [128, HPC, QC], BF16, isOutput=False)
    # bf16 partial outputs: host sums 4 partials per batch in f32; the extra
    # ~0.2% fro error is well within the 2e-2 budget and halves output DMA
    out = nc.declare_dram_parameter("out", [S, HID], BF16, isOutput=True)

    Exp = mybir.ActivationFunctionType.Exp

    with tile.TileContext(nc) as tc:
        with (
            tc.tile_pool(name="consts", bufs=1) as consts,
            tc.tile_pool(name="persist", bufs=1) as persist,
            tc.tile_pool(name="esb", bufs=1) as esb,
            tc.tile_pool(name="ps", bufs=1, space="PSUM") as ps,
        ):
            # ---- DMA issue is spread across engine DGE queues so the Sync
            # engine's serial ~0.7us-per-issue cost doesn't delay startup ----
            xts = persist.tile([128, NSC1, NHT, SC1], BF16)
            # sc0 in 4 sub-chunks so the K projection can start on the first
            for t4 in range(4):
                nc.sync.dma_start(
                    xts[:, 0, 4 * t4 : 4 * t4 + 4, :],
                    xT[:, 0, 4 * t4 : 4 * t4 + 4, :],
                )
            wq_sb = persist.tile([128, NHT, DPH], BF16)
            nc.sync.dma_start(wq_sb[:, 0:8], wq[:, 0:8])
            nc.sync.dma_start(wq_sb[:, 8:16], wq[:, 8:16])
            wo_sb = persist.tile([128, HPC, HID], BF16)
            nc.sync.dma_start(wo_sb[:], wo[:])
            for sc in range(1, NSC1):
                nc.sync.dma_start(xts[:, sc], xT[:, sc])

            # small/early params on other engines' queues
            wk_sb = consts.tile([128, NHT, D], BF16)
            nc.gpsimd.dma_start(wk_sb[:], wk[:])
            wv_sb = consts.tile([128, NHT, D], BF16)
            nc.gpsimd.dma_start(wv_sb[:], wv[:])
            bq_sb = consts.tile([128, HPC], F32)
            nc.scalar.dma_start(bq_sb[:], bq[:])
            bkv_sb = consts.tile([128, 2], F32)
            nc.scalar.dma_start(bkv_sb[:], bkv[:])
            padb_sb = consts.tile([128, NT], F32)
            nc.scalar.dma_start(padb_sb[:], padb[:])
            mask_sb = consts.tile([128, HPC, QC], BF16)
            nc.scalar.dma_start(mask_sb[:], mask4[:])
            ident = consts.tile([128, 128], BF16)
            make_identity(nc, ident[:])
            ones128 = consts.tile([128, 128], BF16)
            nc.vector.memset(ones128[:], 1.0)

            # ---- persistent activations ----
            KT = persist.tile([128, S], BF16)         # K^T [d, kv]
            V = persist.tile([128, NT, 128], BF16)    # V tiles [kv_p, kt, d]
            QT = persist.tile([128, HPC, S], BF16)    # Q^T [d, h, q]
            OT = persist.tile([128, HPC, S], BF16)    # normalized (exp S)V ^T

            def stage1(sc):
                # K chunk
                psk = ps.tile([128, SC1], F32, tag="bg", bufs=2, name="psk")
                for ht in range(NHT):
                    nc.tensor.matmul(
                        psk[:], wk_sb[:, ht, :], xts[:, sc, ht, :],
                        start=(ht == 0), stop=(ht == NHT - 1),
                    )
                nc.vector.tensor_scalar_add(
                    KT[:, ts(sc, SC1)], psk[:], bkv_sb[:, 0:1]
                )
                # V chunk
                psv = ps.tile([128, SC1], F32, tag="bg", bufs=2, name="psv")
                for ht in range(NHT):
                    nc.tensor.matmul(
                        psv[:], wv_sb[:, ht, :], xts[:, sc, ht, :],
                        start=(ht == 0), stop=(ht == NHT - 1),
                    )
                vt_s = esb.tile([128, SC1], BF16, tag="vt", bufs=2, name="vt")
                nc.vector.tensor_scalar_add(vt_s[:], psv[:], bkv_sb[:, 1:2])
                # Q heads
                for dt in range(HPC):
                    psq = ps.tile(
                        [128, SC1], F32, tag="bg", bufs=2, name=f"psq{dt}"
                    )
                    for ht in range(NHT):
                        nc.tensor.matmul(
                            psq[:], wq_sb[:, ht, ts(dt, 128)],
                            xts[:, sc, ht, :],
                            start=(ht == 0), stop=(ht == NHT - 1),
                        )
                    nc.vector.tensor_scalar_add(
                        QT[:, dt, ts(sc, SC1)], psq[:], bq_sb[:, dt : dt + 1]
                    )
                # V transposes last: vt_s is long since drained by now, so
                # these never block the in-order PE queue
                pstr = ps.tile([128, 4, 128], BF16, tag="bg", bufs=2, name="pstr")
                for j in range(4):
                    nc.tensor.transpose(
                        pstr[:, j, :], vt_s[:, ts(j, 128)], ident[:]
                    )
                nc.scalar.copy(V[:, 4 * sc : 4 * sc + 4, :], pstr[:])

            def outproj(qc):
                # out rows [qc*128, (qc+1)*128), all HID cols
                for hc in range(HID // SC1):
                    ps3 = ps.tile(
                        [128, SC1], F32, tag="bg", bufs=2, name=f"ps3_{hc}"
                    )
                    for dt in range(HPC):
                        nc.tensor.matmul(
                            ps3[:],
                            OT[:, dt, ts(qc, QC)],
                            wo_sb[:, dt, ts(hc, SC1)],
                            start=(dt == 0), stop=(dt == HPC - 1),
                        )
                    ot = esb.tile([128, SC1], BF16, tag="out", bufs=3, name="ot")
                    nc.vector.tensor_scalar_add(ot[:], ps3[:], 0.0)
                    nc.sync.dma_start(out[ts(qc, QC), ts(hc, SC1)], ot[:])

            def attn(qc, interleave=None):
                nkt = qc + 1
                pso = ps.tile([128, HPC, QC], F32, tag="o", bufs=2, name="pso")
                psr = ps.tile([128, HPC, QC], F32, tag="r", bufs=1, name="psr")
                pend = []

                def consume():
                    es, kt = pend.pop(0)
                    nc.tensor.matmul(
                        pso[:], V[:, kt, :], es[:],
                        start=(kt == 0), stop=(kt == nkt - 1),
                    )
                    nc.tensor.matmul(
                        psr[:], ones128[:], es[:],
                        start=(kt == 0), stop=(kt == nkt - 1),
                    )

                for kt in range(nkt):
                    psS = ps.tile(
                        [128, HPC, QC], F32, tag="s", bufs=3, name="psS"
                    )
                    nc.tensor.matmul(
                        psS[:], KT[:, ts(kt, 128)], QT[:, :, ts(qc, QC)],
                        start=True, stop=True,
                    )
                    es = esb.tile([128, HPC, QC], BF16, tag="es", bufs=4, name="es")
                    nc.scalar.activation(
                        es[:], psS[:], Exp,
                        bias=padb_sb[:, kt : kt + 1], scale=SCALE,
                    )
                    if kt == qc:
                        nc.vector.tensor_mul(es[:], es[:], mask_sb[:])
                    if pend:
                        consume()
                    pend.append((es, kt))
                # filler PE work while the last exp drains
                if interleave is not None:
                    interleave()
                while pend:
                    consume()
                # normalize: OT[:, h, qc block] = pso * 1/psr
                rec = esb.tile([128, HPC, QC], F32, tag="rec", bufs=2, name="rec")
                nc.vector.reciprocal_approx_fast(rec[:], psr[:])
                nc.vector.tensor_mul(OT[:, :, ts(qc, QC)], pso[:], rec[:])

            # ---------------- main schedule ----------------
            # outproj(qc-1) is PE filler inside attn(qc); across a stage1
            # boundary it instead runs right after stage1 so the PE has work
            # while the DVE drains QT/KT for the next attention chunk.
            for sc in range(NSC1):
                stage1(sc)
                if sc > 0:
                    outproj(4 * sc - 1)
                for qc in range(4 * sc, 4 * sc + 4):
                    filler = None
                    if qc % 4 != 0:
                        filler = lambda q=qc: outproj(q - 1)
                    attn(qc, interleave=filler)
            outproj(NQC - 1)

    nc.compile()
    return nc


def _get_program():
    global _PROGRAM
    if _PROGRAM is None:
        _PROGRAM = _build_program()
    return _PROGRAM


def kernel(**inputs):
    global LAST_RESULT
    hs = np.ascontiguousarray(inputs["hidden_states"], dtype=np.float32)
    pad = np.ascontiguousarray(inputs["padding_mask"], dtype=np.float32)
    Wq = np.asarray(inputs["Wq"], dtype=np.float32)
    Wk = np.asarray(inputs["Wk"], dtype=np.float32)
    Wv = np.asarray(inputs["Wv"], dtype=np.float32)
    Wo = np.asarray(inputs["Wo"], dtype=np.float32)
    bq_v = np.asarray(inputs["bq"], dtype=np.float32)
    bk_v = np.asarray(inputs["bk"], dtype=np.float32)
    bv_v = np.asarray(inputs["bv"], dtype=np.float32)
    bo_v = np.asarray(inputs["bo"], dtype=np.float32)

    # x^T pre-shuffled to [p, sc, ht, c]: partition lines are 16KB contiguous
    xTs = [
        np.ascontiguousarray(
            hs[b].T.reshape(NHT, 128, NSC1, SC1).transpose(1, 2, 0, 3)
        ).astype(NP_BF16)
        for b in range(B)
    ]
    WqT = Wq.T  # [HID, HID]
    # [p, ht, d] shuffles
    WkT = np.ascontiguousarray(
        Wk.T.reshape(NHT, 128, D).transpose(1, 0, 2)
    ).astype(NP_BF16)
    WvT = np.ascontiguousarray(
        Wv.T.reshape(NHT, 128, D).transpose(1, 0, 2)
    ).astype(NP_BF16)
    WoT = Wo.T  # [HID, HID]

    # causal 0/1 mask for the diagonal tile, [128 kv, h, 128 q]
    p_i = np.arange(128)[:, None]
    q_i = np.arange(QC)[None, :]
    m = (q_i >= p_i).astype(np.float32)
    mask4 = np.ascontiguousarray(
        np.broadcast_to(m[:, None, :], (128, HPC, QC))
    ).astype(NP_BF16)

    padbs = [
        np.ascontiguousarray((NEG * pad[b]).reshape(NT, 128).T) for b in range(B)
    ]
    bqs = [
        np.ascontiguousarray(
            bq_v[hg * DPH : (hg + 1) * DPH].reshape(HPC, 128).T
        )
        for hg in range(HPC)
    ]
    bkv = np.ascontiguousarray(np.stack([bk_v, bv_v], axis=1))  # [128, 2]

    nc = _get_program()
    in_maps = []
    for c in range(NCORES):
        b, hg = c // 4, c % 4
        in_maps.append(
            {
                "xT": xTs[b],
                "wq": np.ascontiguousarray(
                    WqT[:, hg * DPH : (hg + 1) * DPH]
                    .reshape(NHT, 128, DPH)
                    .transpose(1, 0, 2)
                ).astype(NP_BF16),
                "wk": WkT,
                "wv": WvT,
                "wo": np.ascontiguousarray(
                    WoT[hg * DPH : (hg + 1) * DPH, :]
                    .reshape(HPC, 128, HID)
                    .transpose(1, 0, 2)
                ).astype(NP_BF16),
                "bq": bqs[hg],
                "bkv": bkv,
                "padb": padbs[b],
                "mask4": mask4,
            }
        )

    LAST_RESULT = run_bass_kernel_spmd(nc, in_maps, list(range(NCORES)))
    res = LAST_RESULT.results

    outp = np.zeros((B, S, HID), np.float32)
    for c in range(NCORES):
        outp[c // 4] += res[c]["out"]
    outp += bo_v[None, None, :]
    return outp


if __name__ == "__main__":
    rng = np.random.default_rng(0)
    demo = {
        "hidden_states": rng.standard_normal((B, S, HID), dtype=np.float32),
        "causal_mask": np.triu(np.ones((1, 1, S, S), np.float32), k=1),
        "padding_mask": np.zeros((B, S), np.float32),
        "Wq": (rng.standard_normal((HID, HID), dtype=np.float32) * 0.02),
        "bq": np.zeros((HID,), np.float32),
        "Wk": (rng.standard_normal((D, HID), dtype=np.float32) * 0.02),
        "bk": np.zeros((D,), np.float32),
        "Wv": (rng.standard_normal((D, HID), dtype=np.float32) * 0.02),
        "bv": np.zeros((D,), np.float32),
        "Wo": (rng.standard_normal((HID, HID), dtype=np.float32) * 0.02),
        "bo": np.zeros((HID,), np.float32),
    }
    o = kernel(**demo)
    print("kernel output", o.shape, o.dtype, float(np.abs(o).mean()))


# revision 15
# speedup vs baseline: 1.6965x; 1.0078x over previous
"""MQA (GQA, 1 KV group) attention kernel for 8 Trainium2 NeuronCores.

Sharding: core c -> batch b = c//4, head-group hg = c%4 (4 of 16 query heads).
Each core computes Q/K/V projections from x[b]^T, causal attention for its 4
heads in transposed layout, and a partial output projection.  Host sums the 4
partials per batch and adds bo.

Schedule is built to keep the PE streaming at its max p-state:
 - attention q-chunks of 128 rows; per kv-tile ONE 4-head-wide scores matmul
   [128kv x 512(h,q)], ONE exp activation, ONE AV matmul, and ONE fused
   rowsum+broadcast matmul (ones^T @ es accumulated in PSUM) -- 3 PE + 1 ACT
   instructions per kv tile, all 512 free columns.
 - causal diag handled by a multiplicative 0/1 bf16 mask on DVE; padding mask
   enters as a per-kv-partition bias in the exp activation.
 - normalization: reciprocal_approx_fast + one DVE mul per chunk.
 - stage-1 bias adds and stage-3 PSUM drains on DVE; exp one-kt-ahead
   software pipeline; outproj(qc-1) interleaved after attn(qc) so the PE has
   filler work while the scalar engine catches up.
"""

import sys

sys.path.insert(0, "/opt/trn_rl_repo")

import ml_dtypes
import numpy as np

import concourse.bass as bass
import concourse.tile as tile
from concourse import bacc
from concourse import mybir
from concourse.bass import ts
from concourse.bass_utils import run_bass_kernel_spmd
from concourse.masks import make_identity

B, S, HID = 2, 2048, 2048
H, D = 16, 128
HPC = 4              # heads per core
DPH = HPC * D        # 512
NCORES = 8
SC1 = 512            # stage-1 s-chunk
NSC1 = S // SC1      # 4
QC = 128             # attention q-chunk
NQC = S // QC        # 16
NT = S // 128        # 16
NHT = HID // 128     # 16
SCALE = 1.0 / float(np.sqrt(D))
NEG = -1.0e9

F32 = mybir.dt.float32
BF16 = mybir.dt.bfloat16
NP_BF16 = ml_dtypes.bfloat16

_PROGRAM = None
LAST_RESULT = None


def _build_program():
    nc = bacc.Bacc()
    # all big inputs pre-shuffled on host so each DMA reads long contiguous
    # per-partition lines (16KB) instead of 1KB strided rows
    xT = nc.declare_dram_parameter("xT", [128, NSC1, NHT, SC1], BF16, isOutput=False)
    wq = nc.declare_dram_parameter("wq", [128, NHT, DPH], BF16, isOutput=False)
    wk = nc.declare_dram_parameter("wk", [128, NHT, D], BF16, isOutput=False)
    wv = nc.declare_dram_parameter("wv", [128, NHT, D], BF16, isOutput=False)
    wo = nc.declare_dram_parameter("wo", [128, HPC, HID], BF16, isOutput=False)
    bq = nc.declare_dram_parameter("bq", [128, HPC], F32, isOutput=False)
    bkv = nc.declare_dram_parameter("bkv", [128, 2], F32, isOutput=False)
    padb = nc.declare_dram_parameter("padb", [128, NT], F32, isOutput=False)
    mask4 = nc.declare_dram_parameter("mask4", [128, HPC, QC], BF16, isOutput=False)
    # bf16 partial outputs: host sums 4 partials per batch in f32; the extra
    # ~0.2% fro error is well within the 2e-2 budget and halves output DMA
    out = nc.declare_dram_parameter("out", [S, HID], BF16, isOutput=True)

    Exp = mybir.ActivationFunctionType.Exp

    with tile.TileContext(nc) as tc:
        with (
            tc.tile_pool(name="consts", bufs=1) as consts,
            tc.tile_pool(name="persist", bufs=1) as persist,
            tc.tile_pool(name="esb", bufs=1) as esb,
            tc.tile_pool(name="ps", bufs=1, space="PSUM") as ps,
        ):
            # ---- DMA issue is spread across engine DGE queues so the Sync
            # engine's serial ~0.7us-per-issue cost doesn't delay startup ----
            # wk/wv first on the fast Sync queue: they gate the first matmul
            wk_sb = consts.tile([128, NHT, D], BF16)
            nc.sync.dma_start(wk_sb[:], wk[:])
            wv_sb = consts.tile([128, NHT, D], BF16)
            nc.sync.dma_start(wv_sb[:], wv[:])
            xts = persist.tile([128, NSC1, NHT, SC1], BF16)
            # sc0 in 4 sub-chunks so the K projection can start on the first
            for t4 in range(4):
                nc.sync.dma_start(
                    xts[:, 0, 4 * t4 : 4 * t4 + 4, :],
                    xT[:, 0, 4 * t4 : 4 * t4 + 4, :],
                )
            wq_sb = persist.tile([128, NHT, DPH], BF16)
            nc.sync.dma_start(wq_sb[:, 0:8], wq[:, 0:8])
            nc.sync.dma_start(wq_sb[:, 8:16], wq[:, 8:16])
            wo_sb = persist.tile([128, HPC, HID], BF16)
            nc.sync.dma_start(wo_sb[:], wo[:])
            for sc in range(1, NSC1):
                nc.sync.dma_start(xts[:, sc], xT[:, sc])

            # small/early params on other engines' queues
            bq_sb = consts.tile([128, HPC], F32)
            nc.scalar.dma_start(bq_sb[:], bq[:])
            bkv_sb = consts.tile([128, 2], F32)
            nc.scalar.dma_start(bkv_sb[:], bkv[:])
            padb_sb = consts.tile([128, NT], F32)
            nc.scalar.dma_start(padb_sb[:], padb[:])
            mask_sb = consts.tile([128, HPC, QC], BF16)
            nc.scalar.dma_start(mask_sb[:], mask4[:])
            ident = consts.tile([128, 128], BF16)
            make_identity(nc, ident[:])
            ones128 = consts.tile([128, 128], BF16)
            nc.vector.memset(ones128[:], 1.0)

            # ---- persistent activations ----
            KT = persist.tile([128, S], BF16)         # K^T [d, kv]
            V = persist.tile([128, NT, 128], BF16)    # V tiles [kv_p, kt, d]
            QT = persist.tile([128, HPC, S], BF16)    # Q^T [d, h, q]
            OT = persist.tile([128, HPC, S], BF16)    # normalized (exp S)V ^T

            def stage1(sc):
                # K chunk
                psk = ps.tile([128, SC1], F32, tag="bg", bufs=2, name="psk")
                for ht in range(NHT):
                    nc.tensor.matmul(
                        psk[:], wk_sb[:, ht, :], xts[:, sc, ht, :],
                        start=(ht == 0), stop=(ht == NHT - 1),
                    )
                nc.vector.tensor_scalar_add(
                    KT[:, ts(sc, SC1)], psk[:], bkv_sb[:, 0:1]
                )
                # V chunk
                psv = ps.tile([128, SC1], F32, tag="bg", bufs=2, name="psv")
                for ht in range(NHT):
                    nc.tensor.matmul(
                        psv[:], wv_sb[:, ht, :], xts[:, sc, ht, :],
                        start=(ht == 0), stop=(ht == NHT - 1),
                    )
                vt_s = esb.tile([128, SC1], BF16, tag="vt", bufs=2, name="vt")
                nc.vector.tensor_scalar_add(vt_s[:], psv[:], bkv_sb[:, 1:2])

                def qhead(dt):
                    psq = ps.tile(
                        [128, SC1], F32, tag="bg", bufs=2, name=f"psq{dt}"
                    )
                    for ht in range(NHT):
                        nc.tensor.matmul(
                            psq[:], wq_sb[:, ht, ts(dt, 128)],
                            xts[:, sc, ht, :],
                            start=(ht == 0), stop=(ht == NHT - 1),
                        )
                    nc.vector.tensor_scalar_add(
                        QT[:, dt, ts(sc, SC1)], psq[:], bq_sb[:, dt : dt + 1]
                    )

                qhead(0)
                qhead(1)
                # transposes mid-sequence: their bg slot reuses Q0's (already
                # drained), and Q3's slot reuse only needs the quick scalar
                # copy, so neither the transposes nor the following attention
                # chunk ever wait on a late DVE drain
                pstr = ps.tile([128, 4, 128], BF16, tag="bg", bufs=2, name="pstr")
                for j in range(4):
                    nc.tensor.transpose(
                        pstr[:, j, :], vt_s[:, ts(j, 128)], ident[:]
                    )
                nc.scalar.copy(V[:, 4 * sc : 4 * sc + 4, :], pstr[:])
                qhead(2)
                qhead(3)

            def outproj(qc):
                # out rows [qc*128, (qc+1)*128), all HID cols; drains
                # alternate DVE/Scalar so neither engine backlogs, and the
                # whole row block ships as a single DMA
                ot = esb.tile([128, HID], BF16, tag="out", bufs=2, name="ot")
                for hc in range(HID // SC1):
                    ps3 = ps.tile(
                        [128, SC1], F32, tag="bg", bufs=2, name=f"ps3_{hc}"
                    )
                    for dt in range(HPC):
                        nc.tensor.matmul(
                            ps3[:],
                            OT[:, dt, ts(qc, QC)],
                            wo_sb[:, dt, ts(hc, SC1)],
                            start=(dt == 0), stop=(dt == HPC - 1),
                        )
                    if hc % 2 == 0:
                        nc.vector.tensor_scalar_add(
                            ot[:, ts(hc, SC1)], ps3[:], 0.0
                        )
                    else:
                        nc.scalar.copy(ot[:, ts(hc, SC1)], ps3[:])
                nc.sync.dma_start(out[ts(qc, QC), :], ot[:])

            def attn(qc, interleave=None):
                nkt = qc + 1
                pso = ps.tile([128, HPC, QC], F32, tag="o", bufs=2, name="pso")
                psr = ps.tile([128, HPC, QC], F32, tag="r", bufs=1, name="psr")
                pend = []

                def consume():
                    es, kt = pend.pop(0)
                    nc.tensor.matmul(
                        pso[:], V[:, kt, :], es[:],
                        start=(kt == 0), stop=(kt == nkt - 1),
                    )
                    nc.tensor.matmul(
                        psr[:], ones128[:], es[:],
                        start=(kt == 0), stop=(kt == nkt - 1),
                    )

                for kt in range(nkt):
                    psS = ps.tile(
                        [128, HPC, QC], F32, tag="s", bufs=3, name="psS"
                    )
                    nc.tensor.matmul(
                        psS[:], KT[:, ts(kt, 128)], QT[:, :, ts(qc, QC)],
                        start=True, stop=True,
                    )
                    es = esb.tile([128, HPC, QC], BF16, tag="es", bufs=4, name="es")
                    nc.scalar.activation(
                        es[:], psS[:], Exp,
                        bias=padb_sb[:, kt : kt + 1], scale=SCALE,
                    )
                    if kt == qc:
                        nc.vector.tensor_mul(es[:], es[:], mask_sb[:])
                    if pend:
                        consume()
                    pend.append((es, kt))
                # filler PE work while the last exp drains
                if interleave is not None:
                    interleave()
                while pend:
                    consume()
                # normalize: OT[:, h, qc block] = pso * 1/psr
                rec = esb.tile([128, HPC, QC], F32, tag="rec", bufs=2, name="rec")
                nc.vector.reciprocal_approx_fast(rec[:], psr[:])
                nc.vector.tensor_mul(OT[:, :, ts(qc, QC)], pso[:], rec[:])

            # ---------------- main schedule ----------------
            # outproj(qc-1) is PE filler inside attn(qc); across a stage1
            # boundary it instead runs right after stage1 so the PE has work
            # while the DVE drains QT/KT for the next attention chunk.
            for sc in range(NSC1):
                stage1(sc)
                if sc > 0:
                    outproj(4 * sc - 1)
                for qc in range(4 * sc, 4 * sc + 4):
                    filler = None
                    if qc % 4 != 0:
                        filler = lambda q=qc: outproj(q - 1)
                    attn(qc, interleave=filler)
            outproj(NQC - 1)

    nc.compile()
    return nc


def _get_program():
    global _PROGRAM
    if _PROGRAM is None:
        _PROGRAM = _build_program()
    return _PROGRAM


def kernel(**inputs):
    global LAST_RESULT
    hs = np.ascontiguousarray(inputs["hidden_states"], dtype=np.float32)
    pad = np.ascontiguousarray(inputs["padding_mask"], dtype=np.float32)
    Wq = np.asarray(inputs["Wq"], dtype=np.float32)
    Wk = np.asarray(inputs["Wk"], dtype=np.float32)
    Wv = np.asarray(inputs["Wv"], dtype=np.float32)
    Wo = np.asarray(inputs["Wo"], dtype=np.float32)
    bq_v = np.asarray(inputs["bq"], dtype=np.float32)
    bk_v = np.asarray(inputs["bk"], dtype=np.float32)
    bv_v = np.asarray(inputs["bv"], dtype=np.float32)
    bo_v = np.asarray(inputs["bo"], dtype=np.float32)

    # x^T pre-shuffled to [p, sc, ht, c]: partition lines are 16KB contiguous
    xTs = [
        np.ascontiguousarray(
            hs[b].T.reshape(NHT, 128, NSC1, SC1).transpose(1, 2, 0, 3)
        ).astype(NP_BF16)
        for b in range(B)
    ]
    WqT = Wq.T  # [HID, HID]
    # [p, ht, d] shuffles
    WkT = np.ascontiguousarray(
        Wk.T.reshape(NHT, 128, D).transpose(1, 0, 2)
    ).astype(NP_BF16)
    WvT = np.ascontiguousarray(
        Wv.T.reshape(NHT, 128, D).transpose(1, 0, 2)
    ).astype(NP_BF16)
    WoT = Wo.T  # [HID, HID]

    # causal 0/1 mask for the diagonal tile, [128 kv, h, 128 q]
    p_i = np.arange(128)[:, None]
    q_i = np.arange(QC)[None, :]
    m = (q_i >= p_i).astype(np.float32)
    mask4 = np.ascontiguousarray(
        np.broadcast_to(m[:, None, :], (128, HPC, QC))
    ).astype(NP_BF16)

    padbs = [
        np.ascontiguousarray((NEG * pad[b]).reshape(NT, 128).T) for b in range(B)
    ]
    bqs = [
        np.ascontiguousarray(
            bq_v[hg * DPH : (hg + 1) * DPH].reshape(HPC, 128).T
        )
        for hg in range(HPC)
    ]
    bkv = np.ascontiguousarray(np.stack([bk_v, bv_v], axis=1))  # [128, 2]

    nc = _get_program()
    in_maps = []
    for c in range(NCORES):
        b, hg = c // 4, c % 4
        in_maps.append(
            {
                "xT": xTs[b],
                "wq": np.ascontiguousarray(
                    WqT[:, hg * DPH : (hg + 1) * DPH]
                    .reshape(NHT, 128, DPH)
                    .transpose(1, 0, 2)
                ).astype(NP_BF16),
                "wk": WkT,
                "wv": WvT,
                "wo": np.ascontiguousarray(
                    WoT[hg * DPH : (hg + 1) * DPH, :]
                    .reshape(HPC, 128, HID)
                    .transpose(1, 0, 2)
                ).astype(NP_BF16),
                "bq": bqs[hg],
                "bkv": bkv,
                "padb": padbs[b],
                "mask4": mask4,
            }
        )

    LAST_RESULT = run_bass_kernel_spmd(nc, in_maps, list(range(NCORES)))
    res = LAST_RESULT.results

    outp = np.zeros((B, S, HID), np.float32)
    for c in range(NCORES):
        outp[c // 4] += res[c]["out"]
    outp += bo_v[None, None, :]
    return outp


if __name__ == "__main__":
    rng = np.random.default_rng(0)
    demo = {
        "hidden_states": rng.standard_normal((B, S, HID), dtype=np.float32),
        "causal_mask": np.triu(np.ones((1, 1, S, S), np.float32), k=1),
        "padding_mask": np.zeros((B, S), np.float32),
        "Wq": (rng.standard_normal((HID, HID), dtype=np.float32) * 0.02),
        "bq": np.zeros((HID,), np.float32),
        "Wk": (rng.standard_normal((D, HID), dtype=np.float32) * 0.02),
        "bk": np.zeros((D,), np.float32),
        "Wv": (rng.standard_normal((D, HID), dtype=np.float32) * 0.02),
        "bv": np.zeros((D,), np.float32),
        "Wo": (rng.standard_normal((HID, HID), dtype=np.float32) * 0.02),
        "bo": np.zeros((HID,), np.float32),
    }
    o = kernel(**demo)
    print("kernel output", o.shape, o.dtype, float(np.abs(o).mean()))


# revision 16
# speedup vs baseline: 1.6991x; 1.0015x over previous
"""MQA (GQA, 1 KV group) attention kernel for 8 Trainium2 NeuronCores.

Sharding: core c -> batch b = c//4, head-group hg = c%4 (4 of 16 query heads).
Each core computes Q/K/V projections from x[b]^T, causal attention for its 4
heads in transposed layout, and a partial output projection.  Host sums the 4
partials per batch and adds bo.

Schedule is built to keep the PE streaming at its max p-state:
 - attention q-chunks of 128 rows; per kv-tile ONE 4-head-wide scores matmul
   [128kv x 512(h,q)], ONE exp activation, ONE AV matmul, and ONE fused
   rowsum+broadcast matmul (ones^T @ es accumulated in PSUM) -- 3 PE + 1 ACT
   instructions per kv tile, all 512 free columns.
 - causal diag handled by a multiplicative 0/1 bf16 mask on DVE; padding mask
   enters as a per-kv-partition bias in the exp activation.
 - normalization: reciprocal_approx_fast + one DVE mul per chunk.
 - stage-1 bias adds and stage-3 PSUM drains on DVE; exp one-kt-ahead
   software pipeline; outproj(qc-1) interleaved after attn(qc) so the PE has
   filler work while the scalar engine catches up.
"""

import sys

sys.path.insert(0, "/opt/trn_rl_repo")

import ml_dtypes
import numpy as np

import concourse.bass as bass
import concourse.tile as tile
from concourse import bacc
from concourse import mybir
from concourse.bass import ts
from concourse.bass_utils import run_bass_kernel_spmd
from concourse.masks import make_identity

B, S, HID = 2, 2048, 2048
H, D = 16, 128
HPC = 4              # heads per core
DPH = HPC * D        # 512
NCORES = 8
SC1 = 512            # stage-1 s-chunk
NSC1 = S // SC1      # 4
QC = 128             # attention q-chunk
NQC = S // QC        # 16
NT = S // 128        # 16
NHT = HID // 128     # 16
SCALE = 1.0 / float(np.sqrt(D))
NEG = -1.0e9

F32 = mybir.dt.float32
BF16 = mybir.dt.bfloat16
NP_BF16 = ml_dtypes.bfloat16

_PROGRAM = None
LAST_RESULT = None


def _build_program():
    nc = bacc.Bacc()
    # all big inputs pre-shuffled on host so each DMA reads long contiguous
    # per-partition lines (16KB) instead of 1KB strided rows
    xT = nc.declare_dram_parameter("xT", [128, NSC1, NHT, SC1], BF16, isOutput=False)
    wq = nc.declare_dram_parameter("wq", [128, NHT, DPH], BF16, isOutput=False)
    wk = nc.declare_dram_parameter("wk", [128, NHT, D], BF16, isOutput=False)
    wv = nc.declare_dram_parameter("wv", [128, NHT, D], BF16, isOutput=False)
    wo = nc.declare_dram_parameter("wo", [128, HPC, HID], BF16, isOutput=False)
    bq = nc.declare_dram_parameter("bq", [128, HPC], F32, isOutput=False)
    bkv = nc.declare_dram_parameter("bkv", [128, 2], F32, isOutput=False)
    padb = nc.declare_dram_parameter("padb", [128, NT], F32, isOutput=False)
    mask4 = nc.declare_dram_parameter("mask4", [128, HPC, QC], BF16, isOutput=False)
    # bf16 partial outputs: host sums 4 partials per batch in f32; the extra
    # ~0.2% fro error is well within the 2e-2 budget and halves output DMA
    out = nc.declare_dram_parameter("out", [S, HID], BF16, isOutput=True)

    Exp = mybir.ActivationFunctionType.Exp

    with tile.TileContext(nc) as tc:
        with (
            tc.tile_pool(name="consts", bufs=1) as consts,
            tc.tile_pool(name="persist", bufs=1) as persist,
            tc.tile_pool(name="esb", bufs=1) as esb,
            tc.tile_pool(name="ps", bufs=1, space="PSUM") as ps,
        ):
            # ---- DMA issue is spread across engine DGE queues so the Sync
            # engine's serial ~0.7us-per-issue cost doesn't delay startup ----
            # wk/wv first on the fast Sync queue: they gate the first matmul
            wk_sb = consts.tile([128, NHT, D], BF16)
            nc.sync.dma_start(wk_sb[:], wk[:])
            wv_sb = consts.tile([128, NHT, D], BF16)
            nc.sync.dma_start(wv_sb[:], wv[:])
            xts = persist.tile([128, NSC1, NHT, SC1], BF16)
            # sc0 in 4 sub-chunks so the K projection can start on the first
            for t4 in range(4):
                nc.sync.dma_start(
                    xts[:, 0, 4 * t4 : 4 * t4 + 4, :],
                    xT[:, 0, 4 * t4 : 4 * t4 + 4, :],
                )
            wq_sb = persist.tile([128, NHT, DPH], BF16)
            nc.sync.dma_start(wq_sb[:, 0:8], wq[:, 0:8])
            nc.sync.dma_start(wq_sb[:, 8:16], wq[:, 8:16])
            wo_sb = persist.tile([128, HPC, HID], BF16)
            nc.sync.dma_start(wo_sb[:], wo[:])
            for sc in range(1, NSC1):
                nc.sync.dma_start(xts[:, sc], xT[:, sc])

            # small/early params on other engines' queues
            bq_sb = consts.tile([128, HPC], F32)
            nc.scalar.dma_start(bq_sb[:], bq[:])
            bkv_sb = consts.tile([128, 2], F32)
            nc.scalar.dma_start(bkv_sb[:], bkv[:])
            padb_sb = consts.tile([128, NT], F32)
            nc.scalar.dma_start(padb_sb[:], padb[:])
            mask_sb = consts.tile([128, HPC, QC], BF16)
            nc.scalar.dma_start(mask_sb[:], mask4[:])
            ident = consts.tile([128, 128], BF16)
            make_identity(nc, ident[:])
            ones128 = consts.tile([128, 128], BF16)
            nc.vector.memset(ones128[:], 1.0)

            # ---- persistent activations ----
            KT = persist.tile([128, S], BF16)         # K^T [d, kv]
            V = persist.tile([128, NT, 128], BF16)    # V tiles [kv_p, kt, d]
            QT = persist.tile([128, HPC, S], BF16)    # Q^T [d, h, q]
            OT = persist.tile([128, HPC, S], BF16)    # normalized (exp S)V ^T

            def stage1(sc):
                # K chunk
                psk = ps.tile([128, SC1], F32, tag="bg", bufs=2, name="psk")
                for ht in range(NHT):
                    nc.tensor.matmul(
                        psk[:], wk_sb[:, ht, :], xts[:, sc, ht, :],
                        start=(ht == 0), stop=(ht == NHT - 1),
                    )
                nc.vector.tensor_scalar_add(
                    KT[:, ts(sc, SC1)], psk[:], bkv_sb[:, 0:1]
                )
                # V chunk
                psv = ps.tile([128, SC1], F32, tag="bg", bufs=2, name="psv")
                for ht in range(NHT):
                    nc.tensor.matmul(
                        psv[:], wv_sb[:, ht, :], xts[:, sc, ht, :],
                        start=(ht == 0), stop=(ht == NHT - 1),
                    )
                vt_s = esb.tile([128, SC1], BF16, tag="vt", bufs=2, name="vt")
                nc.vector.tensor_scalar_add(vt_s[:], psv[:], bkv_sb[:, 1:2])

                def qhead(dt):
                    psq = ps.tile(
                        [128, SC1], F32, tag="bg", bufs=2, name=f"psq{dt}"
                    )
                    for ht in range(NHT):
                        nc.tensor.matmul(
                            psq[:], wq_sb[:, ht, ts(dt, 128)],
                            xts[:, sc, ht, :],
                            start=(ht == 0), stop=(ht == NHT - 1),
                        )
                    nc.vector.tensor_scalar_add(
                        QT[:, dt, ts(sc, SC1)], psq[:], bq_sb[:, dt : dt + 1]
                    )

                qhead(0)
                qhead(1)
                # transposes mid-sequence: their bg slot reuses Q0's (already
                # drained), and Q3's slot reuse only needs the quick scalar
                # copy, so neither the transposes nor the following attention
                # chunk ever wait on a late DVE drain
                pstr = ps.tile([128, 4, 128], BF16, tag="bg", bufs=2, name="pstr")
                for j in range(4):
                    nc.tensor.transpose(
                        pstr[:, j, :], vt_s[:, ts(j, 128)], ident[:]
                    )
                nc.scalar.copy(V[:, 4 * sc : 4 * sc + 4, :], pstr[:])
                qhead(2)
                qhead(3)

            def outproj(qc):
                # out rows [qc*128, (qc+1)*128), all HID cols; drains
                # alternate DVE/Scalar so neither engine backlogs, and the
                # whole row block ships as a single DMA
                ot = esb.tile([128, HID], BF16, tag="out", bufs=2, name="ot")
                for hc in range(HID // SC1):
                    ps3 = ps.tile(
                        [128, SC1], F32, tag="bg", bufs=2, name=f"ps3_{hc}"
                    )
                    for dt in range(HPC):
                        nc.tensor.matmul(
                            ps3[:],
                            OT[:, dt, ts(qc, QC)],
                            wo_sb[:, dt, ts(hc, SC1)],
                            start=(dt == 0), stop=(dt == HPC - 1),
                        )
                    if hc % 2 == 0:
                        nc.vector.tensor_scalar_add(
                            ot[:, ts(hc, SC1)], ps3[:], 0.0
                        )
                    else:
                        nc.scalar.copy(ot[:, ts(hc, SC1)], ps3[:])
                nc.sync.dma_start(out[ts(qc, QC), :], ot[:])

            def attn(qc, interleave=None):
                nkt = qc + 1
                pso = ps.tile([128, HPC, QC], F32, tag="o", bufs=2, name="pso")
                psr = ps.tile([128, HPC, QC], F32, tag="r", bufs=1, name="psr")
                pend = []

                def consume():
                    es, kt = pend.pop(0)
                    nc.tensor.matmul(
                        pso[:], V[:, kt, :], es[:],
                        start=(kt == 0), stop=(kt == nkt - 1),
                    )
                    nc.tensor.matmul(
                        psr[:], ones128[:], es[:],
                        start=(kt == 0), stop=(kt == nkt - 1),
                    )

                for kt in range(nkt):
                    psS = ps.tile(
                        [128, HPC, QC], F32, tag="s", bufs=3, name="psS"
                    )
                    nc.tensor.matmul(
                        psS[:], KT[:, ts(kt, 128)], QT[:, :, ts(qc, QC)],
                        start=True, stop=True,
                    )
                    es = esb.tile([128, HPC, QC], BF16, tag="es", bufs=4, name="es")
                    nc.scalar.activation(
                        es[:], psS[:], Exp,
                        bias=padb_sb[:, kt : kt + 1], scale=SCALE,
                    )
                    if kt == qc:
                        nc.vector.tensor_mul(es[:], es[:], mask_sb[:])
                    # run the PE three scores ahead of AV so the AV never
                    # waits on the ~0.9us scores->exp latency at chunk starts
                    if len(pend) >= 3:
                        consume()
                    pend.append((es, kt))
                # filler PE work while the last exp drains
                if interleave is not None:
                    interleave()
                while pend:
                    consume()
                # normalize: OT[:, h, qc block] = pso * 1/psr
                rec = esb.tile([128, HPC, QC], F32, tag="rec", bufs=2, name="rec")
                nc.vector.reciprocal_approx_fast(rec[:], psr[:])
                nc.vector.tensor_mul(OT[:, :, ts(qc, QC)], pso[:], rec[:])

            # ---------------- main schedule ----------------
            # outproj(qc-1) is PE filler inside attn(qc); across a stage1
            # boundary it instead runs right after stage1 so the PE has work
            # while the DVE drains QT/KT for the next attention chunk.
            for sc in range(NSC1):
                stage1(sc)
                if sc > 0:
                    outproj(4 * sc - 1)
                for qc in range(4 * sc, 4 * sc + 4):
                    filler = None
                    if qc % 4 != 0:
                        filler = lambda q=qc: outproj(q - 1)
                    attn(qc, interleave=filler)
            outproj(NQC - 1)

    nc.compile()
    return nc


def _get_program():
    global _PROGRAM
    if _PROGRAM is None:
        _PROGRAM = _build_program()
    return _PROGRAM


def kernel(**inputs):
    global LAST_RESULT
    hs = np.ascontiguousarray(inputs["hidden_states"], dtype=np.float32)
    pad = np.ascontiguousarray(inputs["padding_mask"], dtype=np.float32)
    Wq = np.asarray(inputs["Wq"], dtype=np.float32)
    Wk = np.asarray(inputs["Wk"], dtype=np.float32)
    Wv = np.asarray(inputs["Wv"], dtype=np.float32)
    Wo = np.asarray(inputs["Wo"], dtype=np.float32)
    bq_v = np.asarray(inputs["bq"], dtype=np.float32)
    bk_v = np.asarray(inputs["bk"], dtype=np.float32)
    bv_v = np.asarray(inputs["bv"], dtype=np.float32)
    bo_v = np.asarray(inputs["bo"], dtype=np.float32)

    # x^T pre-shuffled to [p, sc, ht, c]: partition lines are 16KB contiguous
    xTs = [
        np.ascontiguousarray(
            hs[b].T.reshape(NHT, 128, NSC1, SC1).transpose(1, 2, 0, 3)
        ).astype(NP_BF16)
        for b in range(B)
    ]
    WqT = Wq.T  # [HID, HID]
    # [p, ht, d] shuffles
    WkT = np.ascontiguousarray(
        Wk.T.reshape(NHT, 128, D).transpose(1, 0, 2)
    ).astype(NP_BF16)
    WvT = np.ascontiguousarray(
        Wv.T.reshape(NHT, 128, D).transpose(1, 0, 2)
    ).astype(NP_BF16)
    WoT = Wo.T  # [HID, HID]

    # causal 0/1 mask for the diagonal tile, [128 kv, h, 128 q]
    p_i = np.arange(128)[:, None]
    q_i = np.arange(QC)[None, :]
    m = (q_i >= p_i).astype(np.float32)
    mask4 = np.ascontiguousarray(
        np.broadcast_to(m[:, None, :], (128, HPC, QC))
    ).astype(NP_BF16)

    padbs = [
        np.ascontiguousarray((NEG * pad[b]).reshape(NT, 128).T) for b in range(B)
    ]
    bqs = [
        np.ascontiguousarray(
            bq_v[hg * DPH : (hg + 1) * DPH].reshape(HPC, 128).T
        )
        for hg in range(HPC)
    ]
    bkv = np.ascontiguousarray(np.stack([bk_v, bv_v], axis=1))  # [128, 2]

    nc = _get_program()
    in_maps = []
    for c in range(NCORES):
        b, hg = c // 4, c % 4
        in_maps.append(
            {
                "xT": xTs[b],
                "wq": np.ascontiguousarray(
                    WqT[:, hg * DPH : (hg + 1) * DPH]
                    .reshape(NHT, 128, DPH)
                    .transpose(1, 0, 2)
                ).astype(NP_BF16),
                "wk": WkT,
                "wv": WvT,
                "wo": np.ascontiguousarray(
                    WoT[hg * DPH : (hg + 1) * DPH, :]
                    .reshape(HPC, 128, HID)
                    .transpose(1, 0, 2)
                ).astype(NP_BF16),
                "bq": bqs[hg],
                "bkv": bkv,
                "padb": padbs[b],
                "mask4": mask4,
            }
        )

    LAST_RESULT = run_bass_kernel_spmd(nc, in_maps, list(range(NCORES)))
    res = LAST_RESULT.results

    outp = np.zeros((B, S, HID), np.float32)
    for c in range(NCORES):
        outp[c // 4] += res[c]["out"]
    outp += bo_v[None, None, :]
    return outp


if __name__ == "__main__":
    rng = np.random.default_rng(0)
    demo = {
        "hidden_states": rng.standard_normal((B, S, HID), dtype=np.float32),
        "causal_mask": np.triu(np.ones((1, 1, S, S), np.float32), k=1),
        "padding_mask": np.zeros((B, S), np.float32),
        "Wq": (rng.standard_normal((HID, HID), dtype=np.float32) * 0.02),
        "bq": np.zeros((HID,), np.float32),
        "Wk": (rng.standard_normal((D, HID), dtype=np.float32) * 0.02),
        "bk": np.zeros((D,), np.float32),
        "Wv": (rng.standard_normal((D, HID), dtype=np.float32) * 0.02),
        "bv": np.zeros((D,), np.float32),
        "Wo": (rng.standard_normal((HID, HID), dtype=np.float32) * 0.02),
        "bo": np.zeros((HID,), np.float32),
    }
    o = kernel(**demo)
    print("kernel output", o.shape, o.dtype, float(np.abs(o).mean()))


# revision 20
# speedup vs baseline: 1.7524x; 1.0313x over previous
"""MQA (GQA, 1 KV group) attention kernel for 8 Trainium2 NeuronCores.

Sharding: core c -> batch b = c//4, head-group hg = c%4 (4 of 16 query heads).
Each core computes Q/K/V projections from x[b]^T, causal attention for its 4
heads in transposed layout, and a partial output projection.  Host sums the 4
partials per batch and adds bo.

Schedule is built to keep the PE streaming at its max p-state:
 - attention q-chunks of 128 rows; per kv-tile ONE 4-head-wide scores matmul
   [128kv x 512(h,q)], ONE exp activation, ONE AV matmul, and ONE fused
   rowsum+broadcast matmul (ones^T @ es accumulated in PSUM) -- 3 PE + 1 ACT
   instructions per kv tile, all 512 free columns.
 - causal diag handled by a multiplicative 0/1 bf16 mask on DVE; padding mask
   enters as a per-kv-partition bias in the exp activation.
 - normalization: reciprocal_approx_fast + one DVE mul per chunk.
 - stage-1 bias adds and stage-3 PSUM drains on DVE; exp one-kt-ahead
   software pipeline; outproj(qc-1) interleaved after attn(qc) so the PE has
   filler work while the scalar engine catches up.
"""

import sys

sys.path.insert(0, "/opt/trn_rl_repo")

import ml_dtypes
import numpy as np

import concourse.bass as bass
import concourse.tile as tile
from concourse import bacc
from concourse import mybir
from concourse.bass import ts
from concourse.bass_utils import run_bass_kernel_spmd
from concourse.masks import make_identity

B, S, HID = 2, 2048, 2048
H, D = 16, 128
HPC = 4              # heads per core
DPH = HPC * D        # 512
NCORES = 8
SC1 = 512            # stage-1 s-chunk
NSC1 = S // SC1      # 4
QC = 128             # attention q-chunk
NQC = S // QC        # 16
NT = S // 128        # 16
NHT = HID // 128     # 16
SCALE = 1.0 / float(np.sqrt(D))
NEG = -1.0e9

F32 = mybir.dt.float32
BF16 = mybir.dt.bfloat16
NP_BF16 = ml_dtypes.bfloat16

_PROGRAM = None
LAST_RESULT = None


def _build_program():
    nc = bacc.Bacc()
    # all big inputs pre-shuffled on host so each DMA reads long contiguous
    # per-partition lines (16KB) instead of 1KB strided rows
    xT = nc.declare_dram_parameter("xT", [128, NSC1, NHT, SC1], BF16, isOutput=False)
    wq = nc.declare_dram_parameter("wq", [128, NHT, DPH], BF16, isOutput=False)
    wk = nc.declare_dram_parameter("wk", [128, NHT, D], BF16, isOutput=False)
    wv = nc.declare_dram_parameter("wv", [128, NHT, D], BF16, isOutput=False)
    wo = nc.declare_dram_parameter("wo", [128, HPC, HID], BF16, isOutput=False)
    bq = nc.declare_dram_parameter("bq", [128, HPC], F32, isOutput=False)
    bkv = nc.declare_dram_parameter("bkv", [128, 2], F32, isOutput=False)
    padb = nc.declare_dram_parameter("padb", [128, NT], F32, isOutput=False)
    mask4 = nc.declare_dram_parameter("mask4", [128, HPC, QC], BF16, isOutput=False)
    # bf16 partial outputs: host sums 4 partials per batch in f32; the extra
    # ~0.2% fro error is well within the 2e-2 budget and halves output DMA
    out = nc.declare_dram_parameter("out", [S, HID], BF16, isOutput=True)

    Exp = mybir.ActivationFunctionType.Exp

    with tile.TileContext(nc) as tc:
        with (
            tc.tile_pool(name="consts", bufs=1) as consts,
            tc.tile_pool(name="persist", bufs=1) as persist,
            tc.tile_pool(name="esb", bufs=1) as esb,
            tc.tile_pool(name="ps", bufs=1, space="PSUM") as ps,
        ):
            # ---- DMA issue is spread across engine DGE queues so the Sync
            # engine's serial ~0.7us-per-issue cost doesn't delay startup ----
            # wk/wv first on the fast Sync queue: they gate the first matmul
            wk_sb = consts.tile([128, NHT, D], BF16)
            nc.sync.dma_start(wk_sb[:], wk[:])
            wv_sb = consts.tile([128, NHT, D], BF16)
            nc.sync.dma_start(wv_sb[:], wv[:])
            xts = persist.tile([128, NSC1, NHT, SC1], BF16)
            # sc0 in 4 sub-chunks so the K projection can start on the first
            for t4 in range(4):
                nc.sync.dma_start(
                    xts[:, 0, 4 * t4 : 4 * t4 + 4, :],
                    xT[:, 0, 4 * t4 : 4 * t4 + 4, :],
                )
            wq_sb = persist.tile([128, NHT, DPH], BF16)
            nc.sync.dma_start(wq_sb[:, 0:8], wq[:, 0:8])
            nc.sync.dma_start(wq_sb[:, 8:16], wq[:, 8:16])
            wo_sb = persist.tile([128, HPC, HID], BF16)
            nc.sync.dma_start(wo_sb[:], wo[:])
            for sc in range(1, NSC1):
                nc.sync.dma_start(xts[:, sc], xT[:, sc])

            # small/early params on other engines' queues
            bq_sb = consts.tile([128, HPC], F32)
            nc.scalar.dma_start(bq_sb[:], bq[:])
            bkv_sb = consts.tile([128, 2], F32)
            nc.scalar.dma_start(bkv_sb[:], bkv[:])
            padb_sb = consts.tile([128, NT], F32)
            nc.scalar.dma_start(padb_sb[:], padb[:])
            mask_sb = consts.tile([128, HPC, QC], BF16)
            nc.scalar.dma_start(mask_sb[:], mask4[:])
            ident = consts.tile([128, 128], BF16)
            make_identity(nc, ident[:])
            ones128 = consts.tile([128, 128], BF16)
            nc.vector.memset(ones128[:], 1.0)

            # ---- persistent activations ----
            KT = persist.tile([128, S], BF16)         # K^T [d, kv]
            V = persist.tile([128, NT, 128], BF16)    # V tiles [kv_p, kt, d]
            QT = persist.tile([128, HPC, S], BF16)    # Q^T [d, h, q]
            OT = persist.tile([128, HPC, S], BF16)    # normalized (exp S)V ^T

            def stage1(sc):
                # K chunk
                psk = ps.tile([128, SC1], F32, tag="bg", bufs=2, name="psk")
                for ht in range(NHT):
                    nc.tensor.matmul(
                        psk[:], wk_sb[:, ht, :], xts[:, sc, ht, :],
                        start=(ht == 0), stop=(ht == NHT - 1),
                    )
                nc.vector.tensor_scalar_add(
                    KT[:, ts(sc, SC1)], psk[:], bkv_sb[:, 0:1]
                )
                # V chunk
                psv = ps.tile([128, SC1], F32, tag="bg", bufs=2, name="psv")
                for ht in range(NHT):
                    nc.tensor.matmul(
                        psv[:], wv_sb[:, ht, :], xts[:, sc, ht, :],
                        start=(ht == 0), stop=(ht == NHT - 1),
                    )
                vt_s = esb.tile([128, SC1], BF16, tag="vt", bufs=2, name="vt")
                nc.vector.tensor_scalar_add(vt_s[:], psv[:], bkv_sb[:, 1:2])

                def qhead(dt):
                    psq = ps.tile(
                        [128, SC1], F32, tag="bg", bufs=2, name=f"psq{dt}"
                    )
                    for ht in range(NHT):
                        nc.tensor.matmul(
                            psq[:], wq_sb[:, ht, ts(dt, 128)],
                            xts[:, sc, ht, :],
                            start=(ht == 0), stop=(ht == NHT - 1),
                        )
                    nc.vector.tensor_scalar_add(
                        QT[:, dt, ts(sc, SC1)], psq[:], bq_sb[:, dt : dt + 1]
                    )

                qhead(0)
                qhead(1)
                # transposes mid-sequence: their bg slot reuses Q0's (already
                # drained), and Q3's slot reuse only needs the quick scalar
                # copy, so neither the transposes nor the following attention
                # chunk ever wait on a late DVE drain
                pstr = ps.tile([128, 4, 128], BF16, tag="bg", bufs=2, name="pstr")
                for j in range(4):
                    nc.tensor.transpose(
                        pstr[:, j, :], vt_s[:, ts(j, 128)], ident[:]
                    )
                nc.scalar.copy(V[:, 4 * sc : 4 * sc + 4, :], pstr[:])
                qhead(2)
                qhead(3)

            # outproj work is queued as closures and woven between attention
            # kv-tiles, so the PE absorbs the scores->exp latency with real
            # work instead of idling (the attn phase alone is exp-paced)
            op_queue = []

            def outproj_enqueue(qc):
                ot = esb.tile([128, HID], BF16, tag="out", bufs=2, name="ot")

                def group(hc):
                    ps3 = ps.tile(
                        [128, SC1], F32, tag="bg", bufs=2, name=f"ps3_{hc}"
                    )
                    for dt in range(HPC):
                        nc.tensor.matmul(
                            ps3[:],
                            OT[:, dt, ts(qc, QC)],
                            wo_sb[:, dt, ts(hc, SC1)],
                            start=(dt == 0), stop=(dt == HPC - 1),
                        )
                    if hc % 2 == 0:
                        nc.vector.tensor_scalar_add(
                            ot[:, ts(hc, SC1)], ps3[:], 0.0
                        )
                    else:
                        nc.scalar.copy(ot[:, ts(hc, SC1)], ps3[:])
                    if hc == HID // SC1 - 1:
                        nc.sync.dma_start(out[ts(qc, QC), :], ot[:])

                for hc in range(HID // SC1):
                    op_queue.append(lambda h=hc: group(h))

            def op_pop(n):
                for _ in range(n):
                    if op_queue:
                        op_queue.pop(0)()

            def attn(qc):
                nkt = qc + 1
                npair = nkt // 2
                pso = ps.tile([128, HPC, QC], F32, tag="o", bufs=2, name="pso")
                psr = ps.tile([128, HPC, QC], F32, tag="r", bufs=1, name="psr")
                pend = []
                pair_pend = []
                rsb_emitted = 0
                nrsb = npair + (nkt % 2)

                def rsb(src):
                    nonlocal rsb_emitted
                    nc.tensor.matmul(
                        psr[:], ones128[:], src[:],
                        start=(rsb_emitted == 0), stop=(rsb_emitted == nrsb - 1),
                    )
                    rsb_emitted += 1

                def consume():
                    es, kt = pend.pop(0)
                    nc.tensor.matmul(
                        pso[:], V[:, kt, :], es[:],
                        start=(kt == 0), stop=(kt == nkt - 1),
                    )
                    # rowsums: pairs of es tiles are summed on DVE (bf16) so
                    # the PE streams each pair once instead of twice
                    pair_pend.append(es)
                    if len(pair_pend) == 2:
                        e0, e1 = pair_pend
                        esp = esb.tile(
                            [128, HPC, QC], BF16, tag="esp", bufs=2, name="esp"
                        )
                        nc.vector.tensor_add(esp[:], e0[:], e1[:])
                        rsb(esp)
                        pair_pend.clear()
                    elif kt == nkt - 1:  # lone last tile (odd nkt)
                        rsb(pair_pend.pop())

                for kt in range(nkt):
                    psS = ps.tile(
                        [128, HPC, QC], F32, tag="s", bufs=3, name="psS"
                    )
                    nc.tensor.matmul(
                        psS[:], KT[:, ts(kt, 128)], QT[:, :, ts(qc, QC)],
                        start=True, stop=True,
                    )
                    es = esb.tile([128, HPC, QC], BF16, tag="es", bufs=4, name="es")
                    nc.scalar.activation(
                        es[:], psS[:], Exp,
                        bias=padb_sb[:, kt : kt + 1], scale=SCALE,
                    )
                    if kt == qc:
                        nc.vector.tensor_mul(es[:], es[:], mask_sb[:])
                    if len(pend) >= 2:
                        consume()
                    pend.append((es, kt))
                    if kt % 2 == 1:
                        op_pop(1)
                while pend:
                    consume()
                op_pop(2)
                # normalize: OT[:, h, qc block] = pso * 1/psr
                rec = esb.tile([128, HPC, QC], F32, tag="rec", bufs=2, name="rec")
                nc.vector.reciprocal_approx_fast(rec[:], psr[:])
                nc.vector.tensor_mul(OT[:, :, ts(qc, QC)], pso[:], rec[:])

            # ---------------- main schedule ----------------
            for sc in range(NSC1):
                stage1(sc)
                op_pop(2)
                for qc in range(4 * sc, 4 * sc + 4):
                    attn(qc)
                    outproj_enqueue(qc)
            while op_queue:
                op_pop(1)

    nc.compile()
    return nc


def _get_program():
    global _PROGRAM
    if _PROGRAM is None:
        _PROGRAM = _build_program()
    return _PROGRAM


def kernel(**inputs):
    global LAST_RESULT
    hs = np.ascontiguousarray(inputs["hidden_states"], dtype=np.float32)
    pad = np.ascontiguousarray(inputs["padding_mask"], dtype=np.float32)
    Wq = np.asarray(inputs["Wq"], dtype=np.float32)
    Wk = np.asarray(inputs["Wk"], dtype=np.float32)
    Wv = np.asarray(inputs["Wv"], dtype=np.float32)
    Wo = np.asarray(inputs["Wo"], dtype=np.float32)
    bq_v = np.asarray(inputs["bq"], dtype=np.float32)
    bk_v = np.asarray(inputs["bk"], dtype=np.float32)
    bv_v = np.asarray(inputs["bv"], dtype=np.float32)
    bo_v = np.asarray(inputs["bo"], dtype=np.float32)

    # x^T pre-shuffled to [p, sc, ht, c]: partition lines are 16KB contiguous
    xTs = [
        np.ascontiguousarray(
            hs[b].T.reshape(NHT, 128, NSC1, SC1).transpose(1, 2, 0, 3)
        ).astype(NP_BF16)
        for b in range(B)
    ]
    WqT = Wq.T  # [HID, HID]
    # [p, ht, d] shuffles
    WkT = np.ascontiguousarray(
        Wk.T.reshape(NHT, 128, D).transpose(1, 0, 2)
    ).astype(NP_BF16)
    WvT = np.ascontiguousarray(
        Wv.T.reshape(NHT, 128, D).transpose(1, 0, 2)
    ).astype(NP_BF16)
    WoT = Wo.T  # [HID, HID]

    # causal 0/1 mask for the diagonal tile, [128 kv, h, 128 q]
    p_i = np.arange(128)[:, None]
    q_i = np.arange(QC)[None, :]
    m = (q_i >= p_i).astype(np.float32)
    mask4 = np.ascontiguousarray(
        np.broadcast_to(m[:, None, :], (128, HPC, QC))
    ).astype(NP_BF16)

    padbs = [
        np.ascontiguousarray((NEG * pad[b]).reshape(NT, 128).T) for b in range(B)
    ]
    bqs = [
        np.ascontiguousarray(
            bq_v[hg * DPH : (hg + 1) * DPH].reshape(HPC, 128).T
        )
        for hg in range(HPC)
    ]
    bkv = np.ascontiguousarray(np.stack([bk_v, bv_v], axis=1))  # [128, 2]

    nc = _get_program()
    in_maps = []
    for c in range(NCORES):
        b, hg = c // 4, c % 4
        in_maps.append(
            {
                "xT": xTs[b],
                "wq": np.ascontiguousarray(
                    WqT[:, hg * DPH : (hg + 1) * DPH]
                    .reshape(NHT, 128, DPH)
                    .transpose(1, 0, 2)
                ).astype(NP_BF16),
                "wk": WkT,
                "wv": WvT,
                "wo": np.ascontiguousarray(
                    WoT[hg * DPH : (hg + 1) * DPH, :]
                    .reshape(HPC, 128, HID)
                    .transpose(1, 0, 2)
                ).astype(NP_BF16),
                "bq": bqs[hg],
                "bkv": bkv,
                "padb": padbs[b],
                "mask4": mask4,
            }
        )

    LAST_RESULT = run_bass_kernel_spmd(nc, in_maps, list(range(NCORES)))
    res = LAST_RESULT.results

    outp = np.zeros((B, S, HID), np.float32)
    for c in range(NCORES):
        outp[c // 4] += res[c]["out"]
    outp += bo_v[None, None, :]
    return outp


if __name__ == "__main__":
    rng = np.random.default_rng(0)
    demo = {
        "hidden_states": rng.standard_normal((B, S, HID), dtype=np.float32),
        "causal_mask": np.triu(np.ones((1, 1, S, S), np.float32), k=1),
        "padding_mask": np.zeros((B, S), np.float32),
        "Wq": (rng.standard_normal((HID, HID), dtype=np.float32) * 0.02),
        "bq": np.zeros((HID,), np.float32),
        "Wk": (rng.standard_normal((D, HID), dtype=np.float32) * 0.02),
        "bk": np.zeros((D,), np.float32),
        "Wv": (rng.standard_normal((D, HID), dtype=np.float32) * 0.02),
        "bv": np.zeros((D,), np.float32),
        "Wo": (rng.standard_normal((HID, HID), dtype=np.float32) * 0.02),
        "bo": np.zeros((HID,), np.float32),
    }
    o = kernel(**demo)
    print("kernel output", o.shape, o.dtype, float(np.abs(o).mean()))
